# revision 2
# baseline (speedup 1.0000x reference)
# Trainium2 Bass kernel for GQA with sliding-window attention (v2).
#
# B=1, T=2048, C=2048, 32 q-heads / 8 kv-heads, d_head=64, RoPE,
# sliding-window causal attention (window=512), output projection.
#
# Sharding: tensor parallel over heads across 8 cores. Core c owns q-heads
# [4c, 4c+4) and kv-head c; computes the partial output
# attn_out_shard @ wo[256c:256(c+1), :] in bf16; host sums the 8 partials.
#
# v2 strategy (vs baseline): all-bf16 data paths with fp32 PSUM accumulation;
# x^T produced by hardware DMA-transpose (no PE transposes, no SBUF staging
# copies); scores computed transposed (ST[tk,tq]) so softmax P needs no
# transposition before PV; PV computes O[tq,d] with row-sums accumulated by
# ones-matmuls so normalization is a per-partition tensor_scalar; the 1/8
# scale is folded into the Q RoPE tables; phases A (proj+rope), B (attention)
# and C (output proj) are emission-interleaved per 512-row superblock so
# PE/ACT/DVE/Pool/DMA all stay busy.

import numpy as np

T = 2048
C = 2048
N_HEADS = 32
N_KV = 8
D = 64
WINDOW = 512
NCORES = 8
HQ = N_HEADS // NCORES          # 4 q heads per core
OQ = HQ * D                     # 256
ROPE_BASE = 10000.0
SCALE = 1.0 / 8.0               # 1/sqrt(64)
NB = T // 128                   # 16 row blocks
NS = T // 512                   # 4 superblocks
WIN = 640                       # max key window width per row block

_cache = {}
_DEBUG = False


def _rope_tables():
    inv = 1.0 / (ROPE_BASE ** (np.arange(0, D, 2, dtype=np.float64) / D))
    t = np.arange(T, dtype=np.float64)
    fr = t[:, None] * inv[None, :]            # [T, 32]
    emb = np.concatenate([fr, fr], axis=1)    # [T, 64]
    cos = np.cos(emb).T                       # [64, T]
    sin = np.sin(emb).T
    sinS = sin.copy()
    sinS[: D // 2] *= -1.0                    # signed sin for rotate_half
    cos2 = np.concatenate([cos, cos], axis=0)     # [128, T] (2 heads/tile)
    sinS2 = np.concatenate([sinS, sinS], axis=0)  # [128, T]
    return cos2, sinS2, cos, sinS


def _perm128():
    p = np.zeros((128, 128), dtype=np.float64)
    for s in range(128):
        blk = (s // 64) * 64
        d = s - blk
        p[s, blk + (d + 32) % 64] = 1.0
    return p


def _masks():
    r = np.arange(128)[:, None]
    c = np.arange(128)[None, :]
    lo = (c <= r).astype(np.float64)   # ST j=0 tile (i>=4): allowed c<=r
    hi = (c >= r).astype(np.float64)   # ST diagonal tile: allowed c>=r
    return lo, hi


def _build():
    import concourse.bacc as bacc
    import concourse.mybir as mybir
    import concourse.tile as tile

    f32 = mybir.dt.float32
    bf16 = mybir.dt.bfloat16
    EXP = mybir.ActivationFunctionType.Exp

    nc = bacc.Bacc("TRN2", target_bir_lowering=False, debug=False,
                   num_devices=NCORES)

    # x^T host-packed as [128, s*8192 + h2*4096 + cc*256 + t2] so each
    # half-superblock of x^T loads with one contiguous DMA.
    x_d = nc.dram_tensor("xtr", [128, T * C // 128], bf16,
                         kind="ExternalInput").ap()
    # host-packed weights: [128, n] layouts so each loads with ONE DMA
    wq_d = nc.dram_tensor("wqr", [128, 16 * OQ], bf16, kind="ExternalInput").ap()
    wkv_d = nc.dram_tensor("wkvr", [128, 16 * 128], bf16,
                           kind="ExternalInput").ap()
    wo_d = nc.dram_tensor("wor", [128, 2 * C], bf16, kind="ExternalInput").ap()
    qtab_d = nc.dram_tensor("qtab", [128, 2 * T], bf16,
                            kind="ExternalInput").ap()
    pmm_d = nc.dram_tensor("pmm", [128, 4 * 128], bf16,
                           kind="ExternalInput").ap()
    out_d = nc.dram_tensor("out", [T, C], bf16, kind="ExternalOutput").ap()
    dbg = {}
    if _DEBUG:
        for nm, shp in [("dQTr0", [128, T]), ("dQTr1", [128, T]),
                        ("dKTr", [128, T]), ("dV", [128, NB * 65]),
                        ("dAttnT0", [128, T]), ("dAttnT1", [128, T])]:
            dbg[nm] = nc.dram_tensor(nm, shp, bf16, kind="ExternalOutput").ap()

    with tile.TileContext(nc) as tc:
        from contextlib import ExitStack
        ctx = ExitStack()
        with ctx:
            const = ctx.enter_context(tc.tile_pool(name="const", bufs=1))
            persist = ctx.enter_context(tc.tile_pool(name="persist", bufs=1))

            tmp = ctx.enter_context(tc.tile_pool(name="tmp", bufs=3))
            sm = ctx.enter_context(tc.tile_pool(name="small", bufs=4))
            sexp = ctx.enter_context(tc.tile_pool(name="sexp", bufs=6))
            outp = ctx.enter_context(tc.tile_pool(name="outp", bufs=3))
            psA = ctx.enter_context(
                tc.tile_pool(name="psA", bufs=2, space="PSUM"))
            psST = ctx.enter_context(
                tc.tile_pool(name="psST", bufs=2, space="PSUM"))
            psO = ctx.enter_context(
                tc.tile_pool(name="psO", bufs=1, space="PSUM"))
            psTr = ctx.enter_context(
                tc.tile_pool(name="psTr", bufs=1, space="PSUM"))

            # ---- constants / weights into SBUF (one DMA each) ----
            from concourse.masks import make_identity

            wq_sb = const.tile([128, 16 * OQ], bf16, tag="wq", name="wq")

            xTr = const.tile([128, T * C // 128], bf16, tag="xTr", name="xTr")

            def fetch_xT(s):
                for h2 in range(2):
                    off = s * 8192 + h2 * 4096
                    nc.sync.dma_start(out=xTr[:, off:off + 4096],
                                      in_=x_d[:, off:off + 4096])

            def xT_slice(s, h2, cc):
                off = s * 8192 + h2 * 4096 + cc * 256
                return xTr[:, off:off + 256]

            nc.sync.dma_start(out=wq_sb[:, 0:4 * OQ], in_=wq_d[:, 0:4 * OQ])
            off0 = 0
            nc.sync.dma_start(out=xTr[:, 0:4096], in_=x_d[:, 0:4096])
            for qq in range(1, 4):
                nc.sync.dma_start(out=wq_sb[:, qq * 4 * OQ:(qq + 1) * 4 * OQ],
                                  in_=wq_d[:, qq * 4 * OQ:(qq + 1) * 4 * OQ])
            nc.sync.dma_start(out=xTr[:, 4096:8192], in_=x_d[:, 4096:8192])
            qtab = const.tile([128, 2 * T], bf16, tag="qtab", name="qtab")
            nc.sync.dma_start(out=qtab[:], in_=qtab_d[:, :])
            wkv_sb = const.tile([128, 16 * 128], bf16, tag="wkv", name="wkv")
            nc.sync.dma_start(out=wkv_sb[:], in_=wkv_d[:, :])
            pmm = const.tile([128, 4 * 128], bf16, tag="pmm", name="pmm")
            nc.sync.dma_start(out=pmm[:], in_=pmm_d[:, :])
            fetch_xT(1)
            wo_sb2 = const.tile([128, 2 * C], bf16, tag="wo", name="wo")
            nc.sync.dma_start(out=wo_sb2[:], in_=wo_d[:, :])

            # wq is pre-scaled by 1/8 on the host, so the K rope tables are
            # just the first 64 rows of the (unscaled) Q tables.
            cosQ, sinQ = qtab[:, 0:T], qtab[:, T:2 * T]
            cosK, sinK = qtab[0:64, 0:T], qtab[0:64, T:2 * T]
            perm = pmm[:, 0:128]
            maskLo = pmm[:, 128:256]
            maskHi = pmm[:, 256:384]
            wo_sb = [wo_sb2[:, 0:C], wo_sb2[:, C:2 * C]]

            identb = pmm[:, 384:512]

            # ---- persistent activations ----
            QTr = [persist.tile([128, T], bf16, tag=f"QTr{hp}", name=f"QTr{hp}")
                   for hp in range(2)]
            KTr = persist.tile([128, T], bf16, tag="KTr", name="KTr")
            # V blocks interleaved with a ones column: [V_b | 1] of width 65
            # per 128-row block, so PV row-sums come from the same matmul.
            V_all = persist.tile([128, NB * 65], bf16, tag="V", name="V")
            nc.vector.memset(V_all[:], 1.0)
            attnT = [persist.tile([128, T], bf16, tag=f"attnT{oc}",
                                  name=f"attnT{oc}") for oc in range(2)]

            def rope(ps, P, dst, cos_t, sin_t, scol):
                # dst = ps*cos + rot(ps)*sinS, written as bf16.
                # rot via perm matmul on PE into a separate PSUM tile so the
                # chain is qraw -> rot -> t2 -> add with t1 off-path.
                qraw = tmp.tile([128, 512], bf16, tag="qraw", name="qraw")
                nc.scalar.copy(qraw[:P, :], ps[:P, :])
                t1 = tmp.tile([128, 512], bf16, tag="rt1", name="rt1")
                nc.gpsimd.tensor_mul(t1[:P, :], qraw[:P, :],
                                     cos_t[:P, scol:scol + 512])
                rot = psO.tile([128, 512], f32, tag="pO", name="pO")
                nc.tensor.matmul(rot[:P, :], lhsT=perm[:P, :P],
                                 rhs=qraw[:P, :], start=True, stop=True)
                t2 = tmp.tile([128, 512], bf16, tag="rt2", name="rt2")
                nc.vector.tensor_mul(t2[:P, :], rot[:P, :],
                                     sin_t[:P, scol:scol + 512])
                nc.vector.tensor_add(dst, t1[:P, :], t2[:P, :])

            def phase_a(s):
                scol = s * 512
                def proj(ps, lhs_of):
                    # contract C in 16 chunks, two 256-wide t-halves so the
                    # first half can start before the second transpose lands
                    for h2 in range(2):
                        for cc in range(16):
                            nc.tensor.matmul(
                                ps[:, h2 * 256:(h2 + 1) * 256],
                                lhsT=lhs_of(cc),
                                rhs=xT_slice(s, h2, cc),
                                start=(cc == 0), stop=(cc == 15))

                # Q projections: 2 head-pair blocks of 128 out dims
                for ob in range(2):
                    ps = psA.tile([128, 512], f32, tag="pA", name="pA")
                    proj(ps, lambda cc: wq_sb[:, cc * OQ + ob * 128:
                                              cc * OQ + (ob + 1) * 128])
                    rope(ps, 128, QTr[ob][:, scol:scol + 512], cosQ, sinQ, scol)
                # K+V packed projection: rows 0:64 = K^T, 64:128 = V^T
                ps = psA.tile([128, 512], f32, tag="pA", name="pA")
                proj(ps, lambda cc: wkv_sb[:, cc * 128:(cc + 1) * 128])
                # V: copy V^T rows out, transpose per 128-block to [t, d]
                vtsb = tmp.tile([64, 512], bf16, tag="vtsb", name="vtsb")
                nc.scalar.copy(vtsb[:], ps[64:128, :])
                for half in range(2):
                    vp = psTr.tile([128, 128], bf16, tag="pTr", name="pTr")
                    for b2 in range(2):
                        b = half * 2 + b2
                        nc.tensor.transpose(
                            vp[:, b2 * 64:(b2 + 1) * 64],
                            vtsb[:, b * 128:(b + 1) * 128], identb[:64, :64])
                    tb0 = s * 4 + half * 2
                    for b2 in range(2):
                        nc.vector.tensor_copy(
                            V_all[:, (tb0 + b2) * 65:(tb0 + b2) * 65 + D],
                            vp[:, b2 * 64:(b2 + 1) * 64])
                # K: rope rows 0:64 then duplicate to 64:128 via a PE
                # identity matmul (partition shift) — avoids DMA-queue latency
                rope(ps, 64, KTr[:64, scol:scol + 512], cosK, sinK, scol)
                kd = psO.tile([128, 512], f32, tag="pO", name="pO")
                nc.tensor.matmul(kd[64:128, :], lhsT=identb[0:64, 0:64],
                                 rhs=KTr[:64, scol:scol + 512],
                                 start=True, stop=True)
                nc.vector.tensor_copy(KTr[64:128, scol:scol + 512],
                                      kd[64:128, :])

            def c_chunk(tb, osb, cr):
                op = psA.tile([128, 512], f32, tag="pA", name="pA")
                for oc in range(2):
                    nc.tensor.matmul(
                        op[:], lhsT=attnT[oc][:, tb * 128:(tb + 1) * 128],
                        rhs=wo_sb[oc][:, cr * 512:(cr + 1) * 512],
                        start=(oc == 0), stop=(oc == 1))
                dst = osb[:, cr * 512:(cr + 1) * 512]
                if CR_ENG[cr] == "v" and tb < 14:
                    nc.vector.tensor_copy(dst, op[:])
                else:
                    nc.scalar.copy(dst, op[:])

            def phase_b(i, tb=None, tb2=None):
                # tb: lagging output-projection row whose (always-ready)
                # matmuls are interleaved into this row's stall windows.
                # tb2: extra row emitted at the end (last-segment drain).
                if tb is not None:
                    osb = outp.tile([128, C], bf16, tag="osb", name="osb")
                b0 = max(0, i - 4)
                nj = min(i, 4) + 1
                w = nj * 128
                st_exp = []
                for h in range(HQ):
                    hp, hh = h // 2, h % 2
                    hoff = hh * 64
                    sp = psST.tile([128, WIN], f32, tag="pST", name="pST")
                    qs = QTr[hp][hoff:hoff + 64, i * 128:(i + 1) * 128]
                    for j in range(nj):
                        nc.tensor.matmul(
                            sp[:, j * 128:(j + 1) * 128],
                            lhsT=KTr[hoff:hoff + 64,
                                     (b0 + j) * 128:(b0 + j + 1) * 128],
                            rhs=qs, start=True, stop=True)
                    se = sexp.tile([128, WIN], bf16, tag="se", name="se")
                    nc.scalar.activation(se[:, 0:w], sp[:, 0:w], EXP)
                    if i >= 4:
                        nc.gpsimd.tensor_mul(se[:, 0:128], se[:, 0:128],
                                             maskLo)
                    nc.gpsimd.tensor_mul(se[:, w - 128:w], se[:, w - 128:w],
                                         maskHi)
                    st_exp.append(se)
                if tb is not None:
                    c_chunk(tb, osb, 0)
                    c_chunk(tb, osb, 1)
                po = psO.tile([128, 512], f32, tag="pO", name="pO")
                # masked tiles (j=0 for i>=4, diagonal) go LAST so the PV
                # group starts as soon as exp lands, while masks apply
                if i >= 4:
                    jorder = [1, 2, 3, 0, 4]
                elif i > 0:
                    jorder = list(range(nj - 1)) + [nj - 1]
                else:
                    jorder = [0]
                for h in range(HQ):
                    se = st_exp[h]
                    for n_, j in enumerate(jorder):
                        nc.tensor.matmul(
                            po[:, h * 65:(h + 1) * 65],
                            lhsT=se[:, j * 128:(j + 1) * 128],
                            rhs=V_all[:, (b0 + j) * 65:(b0 + j + 1) * 65],
                            start=(n_ == 0), stop=(n_ == nj - 1),
                            skip_group_check=True)
                if tb is not None:
                    c_chunk(tb, osb, 2)
                    c_chunk(tb, osb, 3)
                    nc.sync.dma_start(out=out_d[tb * 128:(tb + 1) * 128, :],
                                      in_=osb[:])
                rc = sm.tile([128, 4], f32, tag="rc", name="rc")
                nc.vector.reciprocal(rc[:], po[:, 64:260:65])
                ob = sm.tile([128, OQ], bf16, tag="obf", name="obf")
                for h in range(HQ):
                    nc.vector.tensor_scalar_mul(
                        ob[:, h * 64:(h + 1) * 64], po[:, h * 65:h * 65 + 64],
                        rc[:, h:h + 1])
                for hp in range(2):
                    tp = psTr.tile([128, 128], bf16, tag="pTr", name="pTr")
                    for hh in range(2):
                        h = hp * 2 + hh
                        nc.tensor.transpose(
                            tp[hh * 64:(hh + 1) * 64, :],
                            ob[:, h * 64:(h + 1) * 64], identb[:])
                    nc.vector.tensor_copy(
                        attnT[hp][:, i * 128:(i + 1) * 128], tp[:])
                if tb2 is not None:
                    for t2_ in tb2:
                        phase_c(t2_)

            CR_ENG = ["v", "a", "v", "v"]

            def phase_c(tb):
                osb = outp.tile([128, C], bf16, tag="osb", name="osb")
                for cr in range(4):
                    op = psA.tile([128, 512], f32, tag="pA", name="pA")
                    for oc in range(2):
                        nc.tensor.matmul(
                            op[:], lhsT=attnT[oc][:, tb * 128:(tb + 1) * 128],
                            rhs=wo_sb[oc][:, cr * 512:(cr + 1) * 512],
                            start=(oc == 0), stop=(oc == 1))
                    dst = osb[:, cr * 512:(cr + 1) * 512]
                    if cr % 2 == 0:
                        nc.vector.tensor_copy(dst, op[:])
                    else:
                        nc.scalar.copy(dst, op[:])
                nc.sync.dma_start(out=out_d[tb * 128:(tb + 1) * 128, :],
                                  in_=osb[:])

            # ================= interleaved schedule =================
            # phase_c lags phase_b by 2 row-blocks and is emitted BEFORE the
            # b-row so its (always-ready) matmuls fill PE stalls.
            for s in range(NS):
                phase_a(s)
                for k in range(4):
                    i = s * 4 + k
                    phase_b(i, tb=i - 2 if i >= 2 else None,
                            tb2=(14,) if i == 15 else None)
                    if k == 1 and 2 <= s + 1 < NS:
                        fetch_xT(s + 1)
            phase_c(15)

            if _DEBUG:
                nc.sync.dma_start(out=dbg["dQTr0"], in_=QTr[0][:])
                nc.sync.dma_start(out=dbg["dQTr1"], in_=QTr[1][:])
                nc.sync.dma_start(out=dbg["dKTr"], in_=KTr[:])
                nc.sync.dma_start(out=dbg["dV"], in_=V_all[:])
                nc.sync.dma_start(out=dbg["dAttnT0"], in_=attnT[0][:])
                nc.sync.dma_start(out=dbg["dAttnT1"], in_=attnT[1][:])

    nc.compile()
    return nc


def _get_nc():
    if "nc" not in _cache:
        _cache["nc"] = _build()
    return _cache["nc"]


def host_inputs(x, wq, wk, wv, wo, c):
    """Pack core c's inputs into the kernel's DRAM layouts (bf16)."""
    import ml_dtypes
    bf = ml_dtypes.bfloat16
    cos2, sinS2, cosk, sinsk = _rope_tables()
    mlo, mhi = _masks()
    perm = _perm128()
    wq_c = np.asarray(wq)[:, c * OQ:(c + 1) * OQ]
    wkv_c = np.concatenate(
        [np.asarray(wk)[:, c * D:(c + 1) * D],
         np.asarray(wv)[:, c * D:(c + 1) * D]], axis=1)
    wo_c = np.asarray(wo)[c * OQ:(c + 1) * OQ, :]
    wq_c = wq_c * SCALE  # fold the 1/sqrt(d) into wq (2^-3: exact in bf16)
    wqr = wq_c.reshape(16, 128, OQ).transpose(1, 0, 2).reshape(128, 16 * OQ)
    wkvr = wkv_c.reshape(16, 128, 128).transpose(1, 0, 2).reshape(128, 16 * 128)
    wor = wo_c.reshape(2, 128, C).transpose(1, 0, 2).reshape(128, 2 * C)
    qtab = np.concatenate([cos2, sinS2], axis=1)
    pmm = np.concatenate([perm, mlo, mhi, np.eye(128)], axis=1)
    return {
        "wqr": np.ascontiguousarray(wqr).astype(bf),
        "wkvr": np.ascontiguousarray(wkvr).astype(bf),
        "wor": np.ascontiguousarray(wor).astype(bf),
        "qtab": np.ascontiguousarray(qtab).astype(bf),
        "pmm": np.ascontiguousarray(pmm).astype(bf),
    }


def kernel(x, wq, wk, wv, wo):
    from concourse.bass_utils import run_bass_kernel_spmd
    import ml_dtypes

    bf = ml_dtypes.bfloat16
    nc = _get_nc()
    x2 = np.asarray(x, dtype=np.float32).reshape(T, C)
    # pack x^T: [p, (s, h2, cc, t2)] = x[s*512 + h2*256 + t2, cc*128 + p]
    xtr = np.ascontiguousarray(
        x2.reshape(NS, 2, 256, 16, 128).transpose(4, 0, 1, 3, 2)
        .reshape(128, T * C // 128)).astype(bf)
    in_maps = []
    for c in range(NCORES):
        m = host_inputs(x, wq, wk, wv, wo, c)
        m["xtr"] = xtr
        in_maps.append(m)
    res = run_bass_kernel_spmd(nc, in_maps, list(range(NCORES)))
    out = np.zeros((T, C), dtype=np.float32)
    for r in res.results:
        out += np.asarray(r["out"], dtype=np.float32)
    return out.reshape(1, T, C)


# revision 3
# speedup vs baseline: 1.0053x; 1.0053x over previous
# Trainium2 Bass kernel for GQA with sliding-window attention (v2).
#
# B=1, T=2048, C=2048, 32 q-heads / 8 kv-heads, d_head=64, RoPE,
# sliding-window causal attention (window=512), output projection.
#
# Sharding: tensor parallel over heads across 8 cores. Core c owns q-heads
# [4c, 4c+4) and kv-head c; computes the partial output
# attn_out_shard @ wo[256c:256(c+1), :] in bf16; host sums the 8 partials.
#
# v2 strategy (vs baseline): all-bf16 data paths with fp32 PSUM accumulation;
# x^T produced by hardware DMA-transpose (no PE transposes, no SBUF staging
# copies); scores computed transposed (ST[tk,tq]) so softmax P needs no
# transposition before PV; PV computes O[tq,d] with row-sums accumulated by
# ones-matmuls so normalization is a per-partition tensor_scalar; the 1/8
# scale is folded into the Q RoPE tables; phases A (proj+rope), B (attention)
# and C (output proj) are emission-interleaved per 512-row superblock so
# PE/ACT/DVE/Pool/DMA all stay busy.

import numpy as np

T = 2048
C = 2048
N_HEADS = 32
N_KV = 8
D = 64
WINDOW = 512
NCORES = 8
HQ = N_HEADS // NCORES          # 4 q heads per core
OQ = HQ * D                     # 256
ROPE_BASE = 10000.0
SCALE = 1.0 / 8.0               # 1/sqrt(64)
NB = T // 128                   # 16 row blocks
NS = T // 512                   # 4 superblocks
WIN = 640                       # max key window width per row block

_cache = {}
_DEBUG = False


def _rope_tables():
    inv = 1.0 / (ROPE_BASE ** (np.arange(0, D, 2, dtype=np.float64) / D))
    t = np.arange(T, dtype=np.float64)
    fr = t[:, None] * inv[None, :]            # [T, 32]
    emb = np.concatenate([fr, fr], axis=1)    # [T, 64]
    cos = np.cos(emb).T                       # [64, T]
    sin = np.sin(emb).T
    sinS = sin.copy()
    sinS[: D // 2] *= -1.0                    # signed sin for rotate_half
    cos2 = np.concatenate([cos, cos], axis=0)     # [128, T] (2 heads/tile)
    sinS2 = np.concatenate([sinS, sinS], axis=0)  # [128, T]
    return cos2, sinS2, cos, sinS


def _perm128():
    p = np.zeros((128, 128), dtype=np.float64)
    for s in range(128):
        blk = (s // 64) * 64
        d = s - blk
        p[s, blk + (d + 32) % 64] = 1.0
    return p


def _masks():
    r = np.arange(128)[:, None]
    c = np.arange(128)[None, :]
    lo = (c <= r).astype(np.float64)   # ST j=0 tile (i>=4): allowed c<=r
    hi = (c >= r).astype(np.float64)   # ST diagonal tile: allowed c>=r
    return lo, hi


def _build():
    import concourse.bacc as bacc
    import concourse.mybir as mybir
    import concourse.tile as tile

    f32 = mybir.dt.float32
    bf16 = mybir.dt.bfloat16
    EXP = mybir.ActivationFunctionType.Exp

    nc = bacc.Bacc("TRN2", target_bir_lowering=False, debug=False,
                   num_devices=NCORES)

    # x^T host-packed as [128, s*8192 + h2*4096 + cc*256 + t2] so each
    # half-superblock of x^T loads with one contiguous DMA.
    x_d = nc.dram_tensor("xtr", [128, T * C // 128], bf16,
                         kind="ExternalInput").ap()
    # host-packed weights: [128, n] layouts so each loads with ONE DMA
    wq_d = nc.dram_tensor("wqr", [128, 16 * OQ], bf16, kind="ExternalInput").ap()
    wkv_d = nc.dram_tensor("wkvr", [128, 16 * 128], bf16,
                           kind="ExternalInput").ap()
    wo_d = nc.dram_tensor("wor", [128, 2 * C], bf16, kind="ExternalInput").ap()
    qtab_d = nc.dram_tensor("qtab", [128, 2 * T], bf16,
                            kind="ExternalInput").ap()
    pmm_d = nc.dram_tensor("pmm", [128, 4 * 128], bf16,
                           kind="ExternalInput").ap()
    out_d = nc.dram_tensor("out", [T, C], bf16, kind="ExternalOutput").ap()
    dbg = {}
    if _DEBUG:
        for nm, shp in [("dQTr0", [128, T]), ("dQTr1", [128, T]),
                        ("dKTr", [128, T]), ("dV", [128, NB * 65]),
                        ("dAttnT0", [128, T]), ("dAttnT1", [128, T])]:
            dbg[nm] = nc.dram_tensor(nm, shp, bf16, kind="ExternalOutput").ap()

    with tile.TileContext(nc) as tc:
        from contextlib import ExitStack
        ctx = ExitStack()
        with ctx:
            const = ctx.enter_context(tc.tile_pool(name="const", bufs=1))
            persist = ctx.enter_context(tc.tile_pool(name="persist", bufs=1))

            tmp = ctx.enter_context(tc.tile_pool(name="tmp", bufs=4))
            sm = ctx.enter_context(tc.tile_pool(name="small", bufs=6))
            sexp = ctx.enter_context(tc.tile_pool(name="sexp", bufs=6))
            outp = ctx.enter_context(tc.tile_pool(name="outp", bufs=4))
            psA = ctx.enter_context(
                tc.tile_pool(name="psA", bufs=2, space="PSUM"))
            psST = ctx.enter_context(
                tc.tile_pool(name="psST", bufs=2, space="PSUM"))
            psO = ctx.enter_context(
                tc.tile_pool(name="psO", bufs=1, space="PSUM"))
            psTr = ctx.enter_context(
                tc.tile_pool(name="psTr", bufs=1, space="PSUM"))

            # ---- constants / weights into SBUF (one DMA each) ----
            from concourse.masks import make_identity

            wq_sb = const.tile([128, 16 * OQ], bf16, tag="wq", name="wq")

            xTr = const.tile([128, T * C // 128], bf16, tag="xTr", name="xTr")

            def fetch_xT(s):
                for h2 in range(2):
                    off = s * 8192 + h2 * 4096
                    nc.sync.dma_start(out=xTr[:, off:off + 4096],
                                      in_=x_d[:, off:off + 4096])

            def xT_slice(s, h2, cc):
                off = s * 8192 + h2 * 4096 + cc * 256
                return xTr[:, off:off + 256]

            nc.sync.dma_start(out=wq_sb[:, 0:4 * OQ], in_=wq_d[:, 0:4 * OQ])
            off0 = 0
            nc.sync.dma_start(out=xTr[:, 0:4096], in_=x_d[:, 0:4096])
            for qq in range(1, 4):
                nc.sync.dma_start(out=wq_sb[:, qq * 4 * OQ:(qq + 1) * 4 * OQ],
                                  in_=wq_d[:, qq * 4 * OQ:(qq + 1) * 4 * OQ])
            nc.sync.dma_start(out=xTr[:, 4096:8192], in_=x_d[:, 4096:8192])
            wkv_sb = const.tile([128, 16 * 128], bf16, tag="wkv", name="wkv")
            nc.sync.dma_start(out=wkv_sb[:], in_=wkv_d[:, :])
            qtab = const.tile([128, 2 * T], bf16, tag="qtab", name="qtab")
            nc.sync.dma_start(out=qtab[:], in_=qtab_d[:, :])
            pmm = const.tile([128, 4 * 128], bf16, tag="pmm", name="pmm")
            nc.sync.dma_start(out=pmm[:], in_=pmm_d[:, :])
            wo_sb2 = const.tile([128, 2 * C], bf16, tag="wo", name="wo")
            nc.sync.dma_start(out=wo_sb2[:, 0:C], in_=wo_d[:, 0:C])
            nc.sync.dma_start(out=wo_sb2[:, C:2 * C], in_=wo_d[:, C:2 * C])
            fetch_xT(1)

            # wq is pre-scaled by 1/8 on the host, so the K rope tables are
            # just the first 64 rows of the (unscaled) Q tables.
            cosQ, sinQ = qtab[:, 0:T], qtab[:, T:2 * T]
            cosK, sinK = qtab[0:64, 0:T], qtab[0:64, T:2 * T]
            perm = pmm[:, 0:128]
            maskLo = pmm[:, 128:256]
            maskHi = pmm[:, 256:384]
            wo_sb = [wo_sb2[:, 0:C], wo_sb2[:, C:2 * C]]

            identb = pmm[:, 384:512]

            # ---- persistent activations ----
            QTr = [persist.tile([128, T], bf16, tag=f"QTr{hp}", name=f"QTr{hp}")
                   for hp in range(2)]
            KTr = persist.tile([128, T], bf16, tag="KTr", name="KTr")
            # V blocks interleaved with a ones column: [V_b | 1] of width 65
            # per 128-row block, so PV row-sums come from the same matmul.
            V_all = persist.tile([128, NB * 65], bf16, tag="V", name="V")
            nc.vector.memset(V_all[:], 1.0)
            attnT = [persist.tile([128, T], bf16, tag=f"attnT{oc}",
                                  name=f"attnT{oc}") for oc in range(2)]

            def rope(ps, P, dst, cos_t, sin_t, scol):
                # dst = ps*cos + rot(ps)*sinS, written as bf16.
                # rot via perm matmul on PE into a separate PSUM tile so the
                # chain is qraw -> rot -> t2 -> add with t1 off-path.
                qraw = tmp.tile([128, 512], bf16, tag="qraw", name="qraw")
                nc.scalar.copy(qraw[:P, :], ps[:P, :])
                t1 = tmp.tile([128, 512], bf16, tag="rt1", name="rt1")
                nc.gpsimd.tensor_mul(t1[:P, :], qraw[:P, :],
                                     cos_t[:P, scol:scol + 512])
                rot = psO.tile([128, 512], f32, tag="pO", name="pO")
                nc.tensor.matmul(rot[:P, :], lhsT=perm[:P, :P],
                                 rhs=qraw[:P, :], start=True, stop=True)
                t2 = tmp.tile([128, 512], bf16, tag="rt2", name="rt2")
                nc.vector.tensor_mul(t2[:P, :], rot[:P, :],
                                     sin_t[:P, scol:scol + 512])
                nc.vector.tensor_add(dst, t1[:P, :], t2[:P, :])

            def phase_a(s):
                scol = s * 512
                def proj(ps, lhs_of):
                    # contract C in 16 chunks, two 256-wide t-halves so the
                    # first half can start before the second transpose lands
                    for h2 in range(2):
                        for cc in range(16):
                            nc.tensor.matmul(
                                ps[:, h2 * 256:(h2 + 1) * 256],
                                lhsT=lhs_of(cc),
                                rhs=xT_slice(s, h2, cc),
                                start=(cc == 0), stop=(cc == 15))

                # Q projections: 2 head-pair blocks of 128 out dims
                for ob in range(2):
                    ps = psA.tile([128, 512], f32, tag="pA", name="pA")
                    proj(ps, lambda cc: wq_sb[:, cc * OQ + ob * 128:
                                              cc * OQ + (ob + 1) * 128])
                    rope(ps, 128, QTr[ob][:, scol:scol + 512], cosQ, sinQ, scol)
                # K+V packed projection: rows 0:64 = K^T, 64:128 = V^T
                ps = psA.tile([128, 512], f32, tag="pA", name="pA")
                proj(ps, lambda cc: wkv_sb[:, cc * 128:(cc + 1) * 128])
                # V: copy V^T rows out, transpose per 128-block to [t, d]
                vtsb = tmp.tile([64, 512], bf16, tag="vtsb", name="vtsb")
                nc.scalar.copy(vtsb[:], ps[64:128, :])
                for half in range(2):
                    vp = psTr.tile([128, 128], bf16, tag="pTr", name="pTr")
                    for b2 in range(2):
                        b = half * 2 + b2
                        nc.tensor.transpose(
                            vp[:, b2 * 64:(b2 + 1) * 64],
                            vtsb[:, b * 128:(b + 1) * 128], identb[:64, :64])
                    tb0 = s * 4 + half * 2
                    for b2 in range(2):
                        nc.vector.tensor_copy(
                            V_all[:, (tb0 + b2) * 65:(tb0 + b2) * 65 + D],
                            vp[:, b2 * 64:(b2 + 1) * 64])
                # K: rope rows 0:64 then duplicate to 64:128 via a PE
                # identity matmul (partition shift) — avoids DMA-queue latency
                rope(ps, 64, KTr[:64, scol:scol + 512], cosK, sinK, scol)
                kd = psO.tile([128, 512], f32, tag="pO", name="pO")
                nc.tensor.matmul(kd[64:128, :], lhsT=identb[0:64, 0:64],
                                 rhs=KTr[:64, scol:scol + 512],
                                 start=True, stop=True)
                nc.vector.tensor_copy(KTr[64:128, scol:scol + 512],
                                      kd[64:128, :])

            def c_chunk(tb, osb, cr):
                op = psA.tile([128, 512], f32, tag="pA", name="pA")
                for oc in range(2):
                    nc.tensor.matmul(
                        op[:], lhsT=attnT[oc][:, tb * 128:(tb + 1) * 128],
                        rhs=wo_sb[oc][:, cr * 512:(cr + 1) * 512],
                        start=(oc == 0), stop=(oc == 1))
                dst = osb[:, cr * 512:(cr + 1) * 512]
                if CR_ENG[cr] == "v" and tb < 14:
                    nc.vector.tensor_copy(dst, op[:])
                else:
                    nc.scalar.copy(dst, op[:])

            def phase_b(i, tb=None, tb2=None):
                # tb: lagging output-projection row whose (always-ready)
                # matmuls are interleaved into this row's stall windows.
                # tb2: extra row emitted at the end (last-segment drain).
                if tb is not None:
                    osb = outp.tile([128, C], bf16, tag="osb", name="osb")
                b0 = max(0, i - 4)
                nj = min(i, 4) + 1
                w = nj * 128
                st_exp = []
                for h in range(HQ):
                    hp, hh = h // 2, h % 2
                    hoff = hh * 64
                    sp = psST.tile([128, WIN], f32, tag="pST", name="pST")
                    qs = QTr[hp][hoff:hoff + 64, i * 128:(i + 1) * 128]
                    for j in range(nj):
                        nc.tensor.matmul(
                            sp[:, j * 128:(j + 1) * 128],
                            lhsT=KTr[hoff:hoff + 64,
                                     (b0 + j) * 128:(b0 + j + 1) * 128],
                            rhs=qs, start=True, stop=True)
                    se = sexp.tile([128, WIN], bf16, tag="se", name="se")
                    nc.scalar.activation(se[:, 0:w], sp[:, 0:w], EXP)
                    if i >= 4:
                        nc.gpsimd.tensor_mul(se[:, 0:128], se[:, 0:128],
                                             maskLo)
                    nc.gpsimd.tensor_mul(se[:, w - 128:w], se[:, w - 128:w],
                                         maskHi)
                    st_exp.append(se)
                if tb is not None:
                    c_chunk(tb, osb, 0)
                    c_chunk(tb, osb, 1)
                po = psO.tile([128, 512], f32, tag="pO", name="pO")
                # masked tiles (j=0 for i>=4, diagonal) go LAST so the PV
                # group starts as soon as exp lands, while masks apply
                if i >= 4:
                    jorder = [1, 2, 3, 0, 4]
                elif i > 0:
                    jorder = list(range(nj - 1)) + [nj - 1]
                else:
                    jorder = [0]
                for h in range(HQ):
                    se = st_exp[h]
                    for n_, j in enumerate(jorder):
                        nc.tensor.matmul(
                            po[:, h * 65:(h + 1) * 65],
                            lhsT=se[:, j * 128:(j + 1) * 128],
                            rhs=V_all[:, (b0 + j) * 65:(b0 + j + 1) * 65],
                            start=(n_ == 0), stop=(n_ == nj - 1),
                            skip_group_check=True)
                rc = sm.tile([128, 4], f32, tag="rc", name="rc")
                nc.vector.reciprocal(rc[:], po[:, 64:260:65])
                ob = sm.tile([128, OQ], bf16, tag="obf", name="obf")
                for h in range(HQ):
                    nc.vector.tensor_scalar_mul(
                        ob[:, h * 64:(h + 1) * 64], po[:, h * 65:h * 65 + 64],
                        rc[:, h:h + 1])
                for hp in range(2):
                    tp = psTr.tile([128, 128], bf16, tag="pTr", name="pTr")
                    for hh in range(2):
                        h = hp * 2 + hh
                        nc.tensor.transpose(
                            tp[hh * 64:(hh + 1) * 64, :],
                            ob[:, h * 64:(h + 1) * 64], identb[:])
                    nc.vector.tensor_copy(
                        attnT[hp][:, i * 128:(i + 1) * 128], tp[:])
                if tb is not None:
                    c_chunk(tb, osb, 2)
                    c_chunk(tb, osb, 3)
                    nc.sync.dma_start(out=out_d[tb * 128:(tb + 1) * 128, :],
                                      in_=osb[:])
                if tb2 is not None:
                    for t2_ in tb2:
                        phase_c(t2_)

            CR_ENG = ["v", "a", "v", "v"]

            def phase_c(tb):
                osb = outp.tile([128, C], bf16, tag="osb", name="osb")
                for cr in range(4):
                    op = psA.tile([128, 512], f32, tag="pA", name="pA")
                    for oc in range(2):
                        nc.tensor.matmul(
                            op[:], lhsT=attnT[oc][:, tb * 128:(tb + 1) * 128],
                            rhs=wo_sb[oc][:, cr * 512:(cr + 1) * 512],
                            start=(oc == 0), stop=(oc == 1))
                    dst = osb[:, cr * 512:(cr + 1) * 512]
                    if cr % 2 == 0:
                        nc.vector.tensor_copy(dst, op[:])
                    else:
                        nc.scalar.copy(dst, op[:])
                nc.sync.dma_start(out=out_d[tb * 128:(tb + 1) * 128, :],
                                  in_=osb[:])

            # ================= interleaved schedule =================
            # phase_c lags phase_b by 2 row-blocks and is emitted BEFORE the
            # b-row so its (always-ready) matmuls fill PE stalls.
            for s in range(NS):
                phase_a(s)
                for k in range(4):
                    i = s * 4 + k
                    phase_b(i, tb=i - 2 if i >= 2 else None,
                            tb2=(14,) if i == 15 else None)
                    if k == 1 and 2 <= s + 1 < NS:
                        fetch_xT(s + 1)
            phase_c(15)

            if _DEBUG:
                nc.sync.dma_start(out=dbg["dQTr0"], in_=QTr[0][:])
                nc.sync.dma_start(out=dbg["dQTr1"], in_=QTr[1][:])
                nc.sync.dma_start(out=dbg["dKTr"], in_=KTr[:])
                nc.sync.dma_start(out=dbg["dV"], in_=V_all[:])
                nc.sync.dma_start(out=dbg["dAttnT0"], in_=attnT[0][:])
                nc.sync.dma_start(out=dbg["dAttnT1"], in_=attnT[1][:])

    nc.compile()
    return nc


def _get_nc():
    if "nc" not in _cache:
        _cache["nc"] = _build()
    return _cache["nc"]


def host_inputs(x, wq, wk, wv, wo, c):
    """Pack core c's inputs into the kernel's DRAM layouts (bf16)."""
    import ml_dtypes
    bf = ml_dtypes.bfloat16
    cos2, sinS2, cosk, sinsk = _rope_tables()
    mlo, mhi = _masks()
    perm = _perm128()
    wq_c = np.asarray(wq)[:, c * OQ:(c + 1) * OQ]
    wkv_c = np.concatenate(
        [np.asarray(wk)[:, c * D:(c + 1) * D],
         np.asarray(wv)[:, c * D:(c + 1) * D]], axis=1)
    wo_c = np.asarray(wo)[c * OQ:(c + 1) * OQ, :]
    wq_c = wq_c * SCALE  # fold the 1/sqrt(d) into wq (2^-3: exact in bf16)
    wqr = wq_c.reshape(16, 128, OQ).transpose(1, 0, 2).reshape(128, 16 * OQ)
    wkvr = wkv_c.reshape(16, 128, 128).transpose(1, 0, 2).reshape(128, 16 * 128)
    wor = wo_c.reshape(2, 128, C).transpose(1, 0, 2).reshape(128, 2 * C)
    qtab = np.concatenate([cos2, sinS2], axis=1)
    pmm = np.concatenate([perm, mlo, mhi, np.eye(128)], axis=1)
    return {
        "wqr": np.ascontiguousarray(wqr).astype(bf),
        "wkvr": np.ascontiguousarray(wkvr).astype(bf),
        "wor": np.ascontiguousarray(wor).astype(bf),
        "qtab": np.ascontiguousarray(qtab).astype(bf),
        "pmm": np.ascontiguousarray(pmm).astype(bf),
    }


def kernel(x, wq, wk, wv, wo):
    from concourse.bass_utils import run_bass_kernel_spmd
    import ml_dtypes

    bf = ml_dtypes.bfloat16
    nc = _get_nc()
    x2 = np.asarray(x, dtype=np.float32).reshape(T, C)
    # pack x^T: [p, (s, h2, cc, t2)] = x[s*512 + h2*256 + t2, cc*128 + p]
    xtr = np.ascontiguousarray(
        x2.reshape(NS, 2, 256, 16, 128).transpose(4, 0, 1, 3, 2)
        .reshape(128, T * C // 128)).astype(bf)
    in_maps = []
    for c in range(NCORES):
        m = host_inputs(x, wq, wk, wv, wo, c)
        m["xtr"] = xtr
        in_maps.append(m)
    res = run_bass_kernel_spmd(nc, in_maps, list(range(NCORES)))
    out = np.zeros((T, C), dtype=np.float32)
    for r in res.results:
        out += np.asarray(r["out"], dtype=np.float32)
    return out.reshape(1, T, C)


# revision 5
# speedup vs baseline: 1.0395x; 1.0340x over previous
# Trainium2 Bass kernel for GQA with sliding-window attention (v2).
#
# B=1, T=2048, C=2048, 32 q-heads / 8 kv-heads, d_head=64, RoPE,
# sliding-window causal attention (window=512), output projection.
#
# Sharding: tensor parallel over heads across 8 cores. Core c owns q-heads
# [4c, 4c+4) and kv-head c; computes the partial output
# attn_out_shard @ wo[256c:256(c+1), :] in bf16; host sums the 8 partials.
#
# v2 strategy (vs baseline): all-bf16 data paths with fp32 PSUM accumulation;
# x^T produced by hardware DMA-transpose (no PE transposes, no SBUF staging
# copies); scores computed transposed (ST[tk,tq]) so softmax P needs no
# transposition before PV; PV computes O[tq,d] with row-sums accumulated by
# ones-matmuls so normalization is a per-partition tensor_scalar; the 1/8
# scale is folded into the Q RoPE tables; phases A (proj+rope), B (attention)
# and C (output proj) are emission-interleaved per 512-row superblock so
# PE/ACT/DVE/Pool/DMA all stay busy.

import numpy as np

T = 2048
C = 2048
N_HEADS = 32
N_KV = 8
D = 64
WINDOW = 512
NCORES = 8
HQ = N_HEADS // NCORES          # 4 q heads per core
OQ = HQ * D                     # 256
ROPE_BASE = 10000.0
SCALE = 1.0 / 8.0               # 1/sqrt(64)
NB = T // 128                   # 16 row blocks
NS = T // 512                   # 4 superblocks
WIN = 640                       # max key window width per row block

_cache = {}
_DEBUG = False


def _rope_tables():
    inv = 1.0 / (ROPE_BASE ** (np.arange(0, D, 2, dtype=np.float64) / D))
    t = np.arange(T, dtype=np.float64)
    fr = t[:, None] * inv[None, :]            # [T, 32]
    emb = np.concatenate([fr, fr], axis=1)    # [T, 64]
    cos = np.cos(emb).T                       # [64, T]
    sin = np.sin(emb).T
    sinS = sin.copy()
    sinS[: D // 2] *= -1.0                    # signed sin for rotate_half
    cos2 = np.concatenate([cos, cos], axis=0)     # [128, T] (2 heads/tile)
    sinS2 = np.concatenate([sinS, sinS], axis=0)  # [128, T]
    return cos2, sinS2, cos, sinS


def _perm128():
    p = np.zeros((128, 128), dtype=np.float64)
    for s in range(128):
        blk = (s // 64) * 64
        d = s - blk
        p[s, blk + (d + 32) % 64] = 1.0
    return p


def _masks():
    r = np.arange(128)[:, None]
    c = np.arange(128)[None, :]
    lo = (c <= r).astype(np.float64)   # ST j=0 tile (i>=4): allowed c<=r
    hi = (c >= r).astype(np.float64)   # ST diagonal tile: allowed c>=r
    return lo, hi


def _build():
    import concourse.bacc as bacc
    import concourse.mybir as mybir
    import concourse.tile as tile

    f32 = mybir.dt.float32
    bf16 = mybir.dt.bfloat16
    EXP = mybir.ActivationFunctionType.Exp

    nc = bacc.Bacc("TRN2", target_bir_lowering=False, debug=False,
                   num_devices=NCORES)

    # x^T host-packed as [128, s*8192 + h2*4096 + cc*256 + t2] so each
    # half-superblock of x^T loads with one contiguous DMA.
    x_d = nc.dram_tensor("xtr", [128, T * C // 128], bf16,
                         kind="ExternalInput").ap()
    # host-packed weights: [128, n] layouts so each loads with ONE DMA
    wq_d = nc.dram_tensor("wqr", [128, 16 * OQ], bf16, kind="ExternalInput").ap()
    wkv_d = nc.dram_tensor("wkvr", [128, 16 * 128], bf16,
                           kind="ExternalInput").ap()
    wo_d = nc.dram_tensor("wor", [128, 2 * C], bf16, kind="ExternalInput").ap()
    qtab_d = nc.dram_tensor("qtab", [128, 2 * T], bf16,
                            kind="ExternalInput").ap()
    pmm_d = nc.dram_tensor("pmm", [128, 4 * 128], bf16,
                           kind="ExternalInput").ap()
    out_d = nc.dram_tensor("out", [T, C], bf16, kind="ExternalOutput").ap()
    dbg = {}
    if _DEBUG:
        for nm, shp in [("dQTr0", [128, T]), ("dQTr1", [128, T]),
                        ("dKTr", [128, T]), ("dV", [128, NB * 65]),
                        ("dAttnT0", [128, T]), ("dAttnT1", [128, T])]:
            dbg[nm] = nc.dram_tensor(nm, shp, bf16, kind="ExternalOutput").ap()

    with tile.TileContext(nc) as tc:
        from contextlib import ExitStack
        ctx = ExitStack()
        with ctx:
            const = ctx.enter_context(tc.tile_pool(name="const", bufs=1))
            persist = ctx.enter_context(tc.tile_pool(name="persist", bufs=1))

            tmp = ctx.enter_context(tc.tile_pool(name="tmp", bufs=4))
            sm = ctx.enter_context(tc.tile_pool(name="small", bufs=6))
            sexp = ctx.enter_context(tc.tile_pool(name="sexp", bufs=6))
            outp = ctx.enter_context(tc.tile_pool(name="outp", bufs=4))
            psA = ctx.enter_context(
                tc.tile_pool(name="psA", bufs=2, space="PSUM"))
            psST = ctx.enter_context(
                tc.tile_pool(name="psST", bufs=2, space="PSUM"))
            psO = ctx.enter_context(
                tc.tile_pool(name="psO", bufs=1, space="PSUM"))
            psTr = ctx.enter_context(
                tc.tile_pool(name="psTr", bufs=1, space="PSUM"))

            # ---- constants / weights into SBUF (one DMA each) ----
            from concourse.masks import make_identity

            wq_sb = const.tile([128, 16 * OQ], bf16, tag="wq", name="wq")

            xTr = const.tile([128, T * C // 128], bf16, tag="xTr", name="xTr")

            def fetch_xT(s):
                for h2 in range(2):
                    off = s * 8192 + h2 * 4096
                    nc.sync.dma_start(out=xTr[:, off:off + 4096],
                                      in_=x_d[:, off:off + 4096])

            def xT_slice(s, h2, cc):
                off = s * 8192 + h2 * 4096 + cc * 256
                return xTr[:, off:off + 256]

            # startup: interleave wq chunks with quarter-loads of x^T(s0)
            nc.sync.dma_start(out=wq_sb[:, 0:4 * OQ], in_=wq_d[:, 0:4 * OQ])
            nc.sync.dma_start(out=xTr[:, 0:2048], in_=x_d[:, 0:2048])
            nc.sync.dma_start(out=wq_sb[:, 4 * OQ:8 * OQ],
                              in_=wq_d[:, 4 * OQ:8 * OQ])
            nc.sync.dma_start(out=xTr[:, 2048:4096], in_=x_d[:, 2048:4096])
            nc.sync.dma_start(out=wq_sb[:, 8 * OQ:12 * OQ],
                              in_=wq_d[:, 8 * OQ:12 * OQ])
            nc.sync.dma_start(out=xTr[:, 4096:6144], in_=x_d[:, 4096:6144])
            nc.sync.dma_start(out=wq_sb[:, 12 * OQ:16 * OQ],
                              in_=wq_d[:, 12 * OQ:16 * OQ])
            nc.sync.dma_start(out=xTr[:, 6144:8192], in_=x_d[:, 6144:8192])
            wkv_sb = const.tile([128, 16 * 128], bf16, tag="wkv", name="wkv")
            nc.sync.dma_start(out=wkv_sb[:], in_=wkv_d[:, :])
            qtab = const.tile([128, 2 * T], bf16, tag="qtab", name="qtab")
            nc.sync.dma_start(out=qtab[:, 0:1024], in_=qtab_d[:, 0:1024])
            pmm = const.tile([128, 4 * 128], bf16, tag="pmm", name="pmm")
            nc.sync.dma_start(out=pmm[:], in_=pmm_d[:, :])
            nc.sync.dma_start(out=qtab[:, 1024:2048], in_=qtab_d[:, 1024:2048])
            wo_sb2 = const.tile([128, 2 * C], bf16, tag="wo", name="wo")
            nc.sync.dma_start(out=wo_sb2[:, 0:C], in_=wo_d[:, 0:C])
            nc.sync.dma_start(out=qtab[:, 2048:3072], in_=qtab_d[:, 2048:3072])
            nc.sync.dma_start(out=wo_sb2[:, C:2 * C], in_=wo_d[:, C:2 * C])
            nc.sync.dma_start(out=qtab[:, 3072:4096], in_=qtab_d[:, 3072:4096])
            fetch_xT(1)

            # wq is pre-scaled by 1/8 on the host, so the K rope tables are
            # just the first 64 rows of the (unscaled) Q tables.
            def tab(P, s):
                # per-superblock packed tables: [s*1024 + (cos 512 | sin 512)]
                return (qtab[0:P, s * 1024:s * 1024 + 512],
                        qtab[0:P, s * 1024 + 512:s * 1024 + 1024])
            perm = pmm[:, 0:128]
            maskLo = pmm[:, 128:256]
            maskHi = pmm[:, 256:384]
            wo_sb = [wo_sb2[:, 0:C], wo_sb2[:, C:2 * C]]

            identb = pmm[:, 384:512]

            # ---- persistent activations ----
            QTr = [persist.tile([128, T], bf16, tag=f"QTr{hp}", name=f"QTr{hp}")
                   for hp in range(2)]
            KTr = persist.tile([128, T], bf16, tag="KTr", name="KTr")
            # V blocks interleaved with a ones column: [V_b | 1] of width 65
            # per 128-row block, so PV row-sums come from the same matmul.
            V_all = persist.tile([128, NB * 65], bf16, tag="V", name="V")
            nc.vector.memset(V_all[:], 1.0)
            attnT = [persist.tile([128, T], bf16, tag=f"attnT{oc}",
                                  name=f"attnT{oc}") for oc in range(2)]

            def rope(ps, P, dst, cos_t, sin_t):
                # dst = ps*cos + rot(ps)*sinS, written as bf16.
                # rot via perm matmul on PE into a separate PSUM tile so the
                # chain is qraw -> rot -> t2 -> add with t1 off-path.
                qraw = tmp.tile([128, 512], bf16, tag="qraw", name="qraw")
                nc.scalar.copy(qraw[:P, :], ps[:P, :])
                t1 = tmp.tile([128, 512], bf16, tag="rt1", name="rt1")
                nc.gpsimd.tensor_mul(t1[:P, :], qraw[:P, :], cos_t)
                rot = psO.tile([128, 512], f32, tag="pO", name="pO")
                nc.tensor.matmul(rot[:P, :], lhsT=perm[:P, :P],
                                 rhs=qraw[:P, :], start=True, stop=True)
                t2 = tmp.tile([128, 512], bf16, tag="rt2", name="rt2")
                nc.vector.tensor_mul(t2[:P, :], rot[:P, :], sin_t)
                nc.vector.tensor_add(dst, t1[:P, :], t2[:P, :])

            def phase_a(s):
                scol = s * 512
                def proj(ps, lhs_of):
                    # contract C in 16 chunks, two 256-wide t-halves so the
                    # first half can start before the second transpose lands
                    for h2 in range(2):
                        for cc in range(16):
                            nc.tensor.matmul(
                                ps[:, h2 * 256:(h2 + 1) * 256],
                                lhsT=lhs_of(cc),
                                rhs=xT_slice(s, h2, cc),
                                start=(cc == 0), stop=(cc == 15))

                # Q projections: 2 head-pair blocks of 128 out dims
                for ob in range(2):
                    ps = psA.tile([128, 512], f32, tag="pA", name="pA")
                    proj(ps, lambda cc: wq_sb[:, cc * OQ + ob * 128:
                                              cc * OQ + (ob + 1) * 128])
                    cq, sq = tab(128, s)
                    rope(ps, 128, QTr[ob][:, scol:scol + 512], cq, sq)
                # K+V packed projection: rows 0:64 = K^T, 64:128 = V^T
                ps = psA.tile([128, 512], f32, tag="pA", name="pA")
                proj(ps, lambda cc: wkv_sb[:, cc * 128:(cc + 1) * 128])
                # V: copy V^T rows out, transpose per 128-block to [t, d]
                vtsb = tmp.tile([64, 512], bf16, tag="vtsb", name="vtsb")
                nc.scalar.copy(vtsb[:], ps[64:128, :])
                for half in range(2):
                    vp = psTr.tile([128, 128], bf16, tag="pTr", name="pTr")
                    for b2 in range(2):
                        b = half * 2 + b2
                        nc.tensor.transpose(
                            vp[:, b2 * 64:(b2 + 1) * 64],
                            vtsb[:, b * 128:(b + 1) * 128], identb[:64, :64])
                    tb0 = s * 4 + half * 2
                    for b2 in range(2):
                        nc.vector.tensor_copy(
                            V_all[:, (tb0 + b2) * 65:(tb0 + b2) * 65 + D],
                            vp[:, b2 * 64:(b2 + 1) * 64])
                # K: rope rows 0:64 then duplicate to 64:128 via a PE
                # identity matmul (partition shift) — avoids DMA-queue latency
                ck, sk = tab(64, s)
                rope(ps, 64, KTr[:64, scol:scol + 512], ck, sk)
                kd = psO.tile([128, 512], f32, tag="pO", name="pO")
                nc.tensor.matmul(kd[64:128, :], lhsT=identb[0:64, 0:64],
                                 rhs=KTr[:64, scol:scol + 512],
                                 start=True, stop=True)
                nc.vector.tensor_copy(KTr[64:128, scol:scol + 512],
                                      kd[64:128, :])

            def c_chunk(tb, osb, cr):
                op = psA.tile([128, 512], f32, tag="pA", name="pA")
                for oc in range(2):
                    nc.tensor.matmul(
                        op[:], lhsT=attnT[oc][:, tb * 128:(tb + 1) * 128],
                        rhs=wo_sb[oc][:, cr * 512:(cr + 1) * 512],
                        start=(oc == 0), stop=(oc == 1))
                dst = osb[:, cr * 512:(cr + 1) * 512]
                if CR_ENG[cr] == "v" and tb < 14:
                    nc.vector.tensor_copy(dst, op[:])
                else:
                    nc.scalar.copy(dst, op[:])

            def phase_b(i, tb=None, tb2=None):
                # tb: lagging output-projection row whose (always-ready)
                # matmuls are interleaved into this row's stall windows.
                # tb2: extra row emitted at the end (last-segment drain).
                if tb is not None:
                    osb = outp.tile([128, C], bf16, tag="osb", name="osb")
                b0 = max(0, i - 4)
                nj = min(i, 4) + 1
                w = nj * 128
                st_exp = {}
                horder = [0, 2, 1, 3]  # hh=0 heads first: no K-dup dependency
                for h in horder:
                    hp, hh = h // 2, h % 2
                    hoff = hh * 64
                    sp = psST.tile([128, WIN], f32, tag="pST", name="pST")
                    qs = QTr[hp][hoff:hoff + 64, i * 128:(i + 1) * 128]
                    for j in range(nj):
                        nc.tensor.matmul(
                            sp[:, j * 128:(j + 1) * 128],
                            lhsT=KTr[hoff:hoff + 64,
                                     (b0 + j) * 128:(b0 + j + 1) * 128],
                            rhs=qs, start=True, stop=True)
                    se = sexp.tile([128, WIN], bf16, tag="se", name="se")
                    nc.scalar.activation(se[:, 0:w], sp[:, 0:w], EXP)
                    if i >= 4:
                        nc.gpsimd.tensor_mul(se[:, 0:128], se[:, 0:128],
                                             maskLo)
                    nc.gpsimd.tensor_mul(se[:, w - 128:w], se[:, w - 128:w],
                                         maskHi)
                    st_exp[h] = se
                if tb is not None:
                    c_chunk(tb, osb, 0)
                    c_chunk(tb, osb, 1)
                po = psO.tile([128, 512], f32, tag="pO", name="pO")
                # masked tiles (j=0 for i>=4, diagonal) go LAST so the PV
                # group starts as soon as exp lands, while masks apply
                if i >= 4:
                    jorder = [1, 2, 3, 0, 4]
                elif i > 0:
                    jorder = list(range(nj - 1)) + [nj - 1]
                else:
                    jorder = [0]
                for h in horder:
                    se = st_exp[h]
                    for n_, j in enumerate(jorder):
                        nc.tensor.matmul(
                            po[:, h * 65:(h + 1) * 65],
                            lhsT=se[:, j * 128:(j + 1) * 128],
                            rhs=V_all[:, (b0 + j) * 65:(b0 + j + 1) * 65],
                            start=(n_ == 0), stop=(n_ == nj - 1),
                            skip_group_check=True)
                rc = sm.tile([128, 4], f32, tag="rc", name="rc")
                nc.vector.reciprocal(rc[:], po[:, 64:260:65])
                ob = sm.tile([128, OQ], bf16, tag="obf", name="obf")
                for h in range(HQ):
                    nc.vector.tensor_scalar_mul(
                        ob[:, h * 64:(h + 1) * 64], po[:, h * 65:h * 65 + 64],
                        rc[:, h:h + 1])
                for hp in range(2):
                    # one 128x128 transpose yields both heads' [d, tq] halves
                    tp = psTr.tile([128, 128], bf16, tag="pTr", name="pTr")
                    nc.tensor.transpose(
                        tp[:], ob[:, hp * 128:(hp + 1) * 128], identb[:])
                    nc.vector.tensor_copy(
                        attnT[hp][:, i * 128:(i + 1) * 128], tp[:])
                if tb is not None:
                    c_chunk(tb, osb, 2)
                    c_chunk(tb, osb, 3)
                    nc.sync.dma_start(out=out_d[tb * 128:(tb + 1) * 128, :],
                                      in_=osb[:])
                if tb2 is not None:
                    for t2_ in tb2:
                        phase_c(t2_)

            CR_ENG = ["v", "a", "v", "v"]

            def phase_c(tb):
                osb = outp.tile([128, C], bf16, tag="osb", name="osb")
                for cr in range(4):
                    op = psA.tile([128, 512], f32, tag="pA", name="pA")
                    for oc in range(2):
                        nc.tensor.matmul(
                            op[:], lhsT=attnT[oc][:, tb * 128:(tb + 1) * 128],
                            rhs=wo_sb[oc][:, cr * 512:(cr + 1) * 512],
                            start=(oc == 0), stop=(oc == 1))
                    dst = osb[:, cr * 512:(cr + 1) * 512]
                    if cr % 2 == 0:
                        nc.vector.tensor_copy(dst, op[:])
                    else:
                        nc.scalar.copy(dst, op[:])
                    nc.sync.dma_start(
                        out=out_d[tb * 128:(tb + 1) * 128,
                                  cr * 512:(cr + 1) * 512], in_=dst)

            # ================= interleaved schedule =================
            # phase_c lags phase_b by 2 row-blocks and is emitted BEFORE the
            # b-row so its (always-ready) matmuls fill PE stalls.
            for s in range(NS):
                phase_a(s)
                for k in range(4):
                    i = s * 4 + k
                    phase_b(i, tb=i - 2 if i >= 2 else None,
                            tb2=(14,) if i == 15 else None)
                    if k == 1 and 2 <= s + 1 < NS:
                        fetch_xT(s + 1)
            phase_c(15)

            if _DEBUG:
                nc.sync.dma_start(out=dbg["dQTr0"], in_=QTr[0][:])
                nc.sync.dma_start(out=dbg["dQTr1"], in_=QTr[1][:])
                nc.sync.dma_start(out=dbg["dKTr"], in_=KTr[:])
                nc.sync.dma_start(out=dbg["dV"], in_=V_all[:])
                nc.sync.dma_start(out=dbg["dAttnT0"], in_=attnT[0][:])
                nc.sync.dma_start(out=dbg["dAttnT1"], in_=attnT[1][:])

    nc.compile()
    return nc


def _get_nc():
    if "nc" not in _cache:
        _cache["nc"] = _build()
    return _cache["nc"]


def host_inputs(x, wq, wk, wv, wo, c):
    """Pack core c's inputs into the kernel's DRAM layouts (bf16)."""
    import ml_dtypes
    bf = ml_dtypes.bfloat16
    cos2, sinS2, cosk, sinsk = _rope_tables()
    mlo, mhi = _masks()
    perm = _perm128()
    wq_c = np.asarray(wq)[:, c * OQ:(c + 1) * OQ]
    wkv_c = np.concatenate(
        [np.asarray(wk)[:, c * D:(c + 1) * D],
         np.asarray(wv)[:, c * D:(c + 1) * D]], axis=1)
    wo_c = np.asarray(wo)[c * OQ:(c + 1) * OQ, :]
    wq_c = wq_c * SCALE  # fold the 1/sqrt(d) into wq (2^-3: exact in bf16)
    wqr = wq_c.reshape(16, 128, OQ).transpose(1, 0, 2).reshape(128, 16 * OQ)
    wkvr = wkv_c.reshape(16, 128, 128).transpose(1, 0, 2).reshape(128, 16 * 128)
    wor = wo_c.reshape(2, 128, C).transpose(1, 0, 2).reshape(128, 2 * C)
    qtab = np.concatenate(
        [np.concatenate([cos2[:, s * 512:(s + 1) * 512],
                         sinS2[:, s * 512:(s + 1) * 512]], axis=1)
         for s in range(NS)], axis=1)
    pmm = np.concatenate([perm, mlo, mhi, np.eye(128)], axis=1)
    return {
        "wqr": np.ascontiguousarray(wqr).astype(bf),
        "wkvr": np.ascontiguousarray(wkvr).astype(bf),
        "wor": np.ascontiguousarray(wor).astype(bf),
        "qtab": np.ascontiguousarray(qtab).astype(bf),
        "pmm": np.ascontiguousarray(pmm).astype(bf),
    }


def kernel(x, wq, wk, wv, wo):
    from concourse.bass_utils import run_bass_kernel_spmd
    import ml_dtypes

    bf = ml_dtypes.bfloat16
    nc = _get_nc()
    x2 = np.asarray(x, dtype=np.float32).reshape(T, C)
    # pack x^T: [p, (s, h2, cc, t2)] = x[s*512 + h2*256 + t2, cc*128 + p]
    xtr = np.ascontiguousarray(
        x2.reshape(NS, 2, 256, 16, 128).transpose(4, 0, 1, 3, 2)
        .reshape(128, T * C // 128)).astype(bf)
    in_maps = []
    for c in range(NCORES):
        m = host_inputs(x, wq, wk, wv, wo, c)
        m["xtr"] = xtr
        in_maps.append(m)
    res = run_bass_kernel_spmd(nc, in_maps, list(range(NCORES)))
    out = np.zeros((T, C), dtype=np.float32)
    for r in res.results:
        out += np.asarray(r["out"], dtype=np.float32)
    return out.reshape(1, T, C)


# revision 6
# speedup vs baseline: 1.0469x; 1.0071x over previous
# Trainium2 Bass kernel for GQA with sliding-window attention (v2).
#
# B=1, T=2048, C=2048, 32 q-heads / 8 kv-heads, d_head=64, RoPE,
# sliding-window causal attention (window=512), output projection.
#
# Sharding: tensor parallel over heads across 8 cores. Core c owns q-heads
# [4c, 4c+4) and kv-head c; computes the partial output
# attn_out_shard @ wo[256c:256(c+1), :] in bf16; host sums the 8 partials.
#
# v2 strategy (vs baseline): all-bf16 data paths with fp32 PSUM accumulation;
# x^T produced by hardware DMA-transpose (no PE transposes, no SBUF staging
# copies); scores computed transposed (ST[tk,tq]) so softmax P needs no
# transposition before PV; PV computes O[tq,d] with row-sums accumulated by
# ones-matmuls so normalization is a per-partition tensor_scalar; the 1/8
# scale is folded into the Q RoPE tables; phases A (proj+rope), B (attention)
# and C (output proj) are emission-interleaved per 512-row superblock so
# PE/ACT/DVE/Pool/DMA all stay busy.

import numpy as np

T = 2048
C = 2048
N_HEADS = 32
N_KV = 8
D = 64
WINDOW = 512
NCORES = 8
HQ = N_HEADS // NCORES          # 4 q heads per core
OQ = HQ * D                     # 256
ROPE_BASE = 10000.0
SCALE = 1.0 / 8.0               # 1/sqrt(64)
NB = T // 128                   # 16 row blocks
NS = T // 512                   # 4 superblocks
WIN = 640                       # max key window width per row block

_cache = {}
_DEBUG = False


def _rope_tables():
    inv = 1.0 / (ROPE_BASE ** (np.arange(0, D, 2, dtype=np.float64) / D))
    t = np.arange(T, dtype=np.float64)
    fr = t[:, None] * inv[None, :]            # [T, 32]
    emb = np.concatenate([fr, fr], axis=1)    # [T, 64]
    cos = np.cos(emb).T                       # [64, T]
    sin = np.sin(emb).T
    sinS = sin.copy()
    sinS[: D // 2] *= -1.0                    # signed sin for rotate_half
    cos2 = np.concatenate([cos, cos], axis=0)     # [128, T] (2 heads/tile)
    sinS2 = np.concatenate([sinS, sinS], axis=0)  # [128, T]
    return cos2, sinS2, cos, sinS


def _perm128():
    p = np.zeros((128, 128), dtype=np.float64)
    for s in range(128):
        blk = (s // 64) * 64
        d = s - blk
        p[s, blk + (d + 32) % 64] = 1.0
    return p


def _masks():
    r = np.arange(128)[:, None]
    c = np.arange(128)[None, :]
    lo = (c <= r).astype(np.float64)   # ST j=0 tile (i>=4): allowed c<=r
    hi = (c >= r).astype(np.float64)   # ST diagonal tile: allowed c>=r
    return lo, hi


def _build():
    import concourse.bacc as bacc
    import concourse.mybir as mybir
    import concourse.tile as tile

    f32 = mybir.dt.float32
    bf16 = mybir.dt.bfloat16
    EXP = mybir.ActivationFunctionType.Exp

    nc = bacc.Bacc("TRN2", target_bir_lowering=False, debug=False,
                   num_devices=NCORES)

    # x^T host-packed as [128, s*8192 + h2*4096 + cc*256 + t2] so each
    # half-superblock of x^T loads with one contiguous DMA.
    x_d = nc.dram_tensor("xtr", [128, T * C // 128], bf16,
                         kind="ExternalInput").ap()
    # host-packed weights: [128, n] layouts so each loads with ONE DMA
    wq_d = nc.dram_tensor("wqr", [128, 16 * OQ], bf16, kind="ExternalInput").ap()
    wkv_d = nc.dram_tensor("wkvr", [128, 16 * 128], bf16,
                           kind="ExternalInput").ap()
    wo_d = nc.dram_tensor("wor", [128, 2 * C], bf16, kind="ExternalInput").ap()
    qtab_d = nc.dram_tensor("qtab", [128, 2 * T], bf16,
                            kind="ExternalInput").ap()
    pmm_d = nc.dram_tensor("pmm", [128, 4 * 128], bf16,
                           kind="ExternalInput").ap()
    out_d = nc.dram_tensor("out", [T, C], bf16, kind="ExternalOutput").ap()
    dbg = {}
    if _DEBUG:
        for nm, shp in [("dQTr0", [128, T]), ("dQTr1", [128, T]),
                        ("dKTr", [128, T]), ("dV", [128, NB * 65]),
                        ("dAttnT0", [128, T]), ("dAttnT1", [128, T])]:
            dbg[nm] = nc.dram_tensor(nm, shp, bf16, kind="ExternalOutput").ap()

    with tile.TileContext(nc) as tc:
        from contextlib import ExitStack
        ctx = ExitStack()
        with ctx:
            const = ctx.enter_context(tc.tile_pool(name="const", bufs=1))
            persist = ctx.enter_context(tc.tile_pool(name="persist", bufs=1))

            tmp = ctx.enter_context(tc.tile_pool(name="tmp", bufs=4))
            sm = ctx.enter_context(tc.tile_pool(name="small", bufs=6))
            sexp = ctx.enter_context(tc.tile_pool(name="sexp", bufs=6))
            outp = ctx.enter_context(tc.tile_pool(name="outp", bufs=4))
            psA = ctx.enter_context(
                tc.tile_pool(name="psA", bufs=2, space="PSUM"))
            psST = ctx.enter_context(
                tc.tile_pool(name="psST", bufs=2, space="PSUM"))
            psO = ctx.enter_context(
                tc.tile_pool(name="psO", bufs=1, space="PSUM"))
            psTr = ctx.enter_context(
                tc.tile_pool(name="psTr", bufs=1, space="PSUM"))

            # ---- constants / weights into SBUF (one DMA each) ----
            from concourse.masks import make_identity

            wq_sb = const.tile([128, 16 * OQ], bf16, tag="wq", name="wq")

            xTr = const.tile([128, T * C // 128], bf16, tag="xTr", name="xTr")

            def fetch_xT(s):
                for h2 in range(2):
                    off = s * 8192 + h2 * 4096
                    nc.sync.dma_start(out=xTr[:, off:off + 4096],
                                      in_=x_d[:, off:off + 4096])

            def xT_slice(s, h2, cc):
                off = s * 8192 + h2 * 4096 + cc * 256
                return xTr[:, off:off + 256]

            # startup: interleave wq chunks with quarter-loads of x^T(s0)
            nc.sync.dma_start(out=wq_sb[:, 0:4 * OQ], in_=wq_d[:, 0:4 * OQ])
            nc.sync.dma_start(out=xTr[:, 0:2048], in_=x_d[:, 0:2048])
            nc.sync.dma_start(out=wq_sb[:, 4 * OQ:8 * OQ],
                              in_=wq_d[:, 4 * OQ:8 * OQ])
            nc.sync.dma_start(out=xTr[:, 2048:4096], in_=x_d[:, 2048:4096])
            nc.sync.dma_start(out=wq_sb[:, 8 * OQ:12 * OQ],
                              in_=wq_d[:, 8 * OQ:12 * OQ])
            nc.sync.dma_start(out=xTr[:, 4096:6144], in_=x_d[:, 4096:6144])
            nc.sync.dma_start(out=wq_sb[:, 12 * OQ:16 * OQ],
                              in_=wq_d[:, 12 * OQ:16 * OQ])
            nc.sync.dma_start(out=xTr[:, 6144:8192], in_=x_d[:, 6144:8192])
            wkv_sb = const.tile([128, 16 * 128], bf16, tag="wkv", name="wkv")
            nc.sync.dma_start(out=wkv_sb[:], in_=wkv_d[:, :])
            qtab = const.tile([128, 2 * T], bf16, tag="qtab", name="qtab")
            nc.sync.dma_start(out=qtab[:, 0:1024], in_=qtab_d[:, 0:1024])
            pmm = const.tile([128, 4 * 128], bf16, tag="pmm", name="pmm")
            nc.sync.dma_start(out=pmm[:], in_=pmm_d[:, :])
            nc.sync.dma_start(out=qtab[:, 1024:2048], in_=qtab_d[:, 1024:2048])
            wo_sb2 = const.tile([128, 2 * C], bf16, tag="wo", name="wo")
            nc.sync.dma_start(out=wo_sb2[:, 0:C], in_=wo_d[:, 0:C])
            nc.sync.dma_start(out=qtab[:, 2048:3072], in_=qtab_d[:, 2048:3072])
            nc.sync.dma_start(out=wo_sb2[:, C:2 * C], in_=wo_d[:, C:2 * C])
            nc.sync.dma_start(out=qtab[:, 3072:4096], in_=qtab_d[:, 3072:4096])
            fetch_xT(1)

            # wq is pre-scaled by 1/8 on the host, so the K rope tables are
            # just the first 64 rows of the (unscaled) Q tables.
            def tab(P, s):
                # per-superblock packed tables: [s*1024 + (cos 512 | sin 512)]
                return (qtab[0:P, s * 1024:s * 1024 + 512],
                        qtab[0:P, s * 1024 + 512:s * 1024 + 1024])
            perm = pmm[:, 0:128]
            maskLo = pmm[:, 128:256]
            maskHi = pmm[:, 256:384]
            wo_sb = [wo_sb2[:, 0:C], wo_sb2[:, C:2 * C]]

            identb = pmm[:, 384:512]

            # ---- persistent activations ----
            QTr = [persist.tile([128, T], bf16, tag=f"QTr{hp}", name=f"QTr{hp}")
                   for hp in range(2)]
            KTr = persist.tile([128, T], bf16, tag="KTr", name="KTr")
            # V blocks interleaved with a ones column: [V_b | 1] of width 65
            # per 128-row block, so PV row-sums come from the same matmul.
            V_all = persist.tile([128, NB * 65], bf16, tag="V", name="V")
            nc.vector.memset(V_all[:], 1.0)
            attnT = [persist.tile([128, T], bf16, tag=f"attnT{oc}",
                                  name=f"attnT{oc}") for oc in range(2)]

            def rope(ps, P, dst, cos_t, sin_t):
                # dst = ps*cos + rot(ps)*sinS, written as bf16.
                # rot via perm matmul on PE into a separate PSUM tile so the
                # chain is qraw -> rot -> t2 -> add with t1 off-path.
                qraw = tmp.tile([128, 512], bf16, tag="qraw", name="qraw")
                nc.scalar.copy(qraw[:P, :], ps[:P, :])
                t1 = tmp.tile([128, 512], bf16, tag="rt1", name="rt1")
                nc.gpsimd.tensor_mul(t1[:P, :], qraw[:P, :], cos_t)
                rot = psO.tile([128, 512], f32, tag="pO", name="pO")
                nc.tensor.matmul(rot[:P, :], lhsT=perm[:P, :P],
                                 rhs=qraw[:P, :], start=True, stop=True)
                t2 = tmp.tile([128, 512], bf16, tag="rt2", name="rt2")
                nc.vector.tensor_mul(t2[:P, :], rot[:P, :], sin_t)
                nc.vector.tensor_add(dst, t1[:P, :], t2[:P, :])

            def phase_a(s):
                scol = s * 512
                def proj(ps, lhs_of):
                    # contract C in 16 chunks, two 256-wide t-halves so the
                    # first half can start before the second transpose lands
                    for h2 in range(2):
                        for cc in range(16):
                            nc.tensor.matmul(
                                ps[:, h2 * 256:(h2 + 1) * 256],
                                lhsT=lhs_of(cc),
                                rhs=xT_slice(s, h2, cc),
                                start=(cc == 0), stop=(cc == 15))

                # Q projections: 2 head-pair blocks of 128 out dims
                for ob in range(2):
                    ps = psA.tile([128, 512], f32, tag="pA", name="pA")
                    proj(ps, lambda cc: wq_sb[:, cc * OQ + ob * 128:
                                              cc * OQ + (ob + 1) * 128])
                    cq, sq = tab(128, s)
                    rope(ps, 128, QTr[ob][:, scol:scol + 512], cq, sq)
                # K+V packed projection: rows 0:64 = K^T, 64:128 = V^T
                ps = psA.tile([128, 512], f32, tag="pA", name="pA")
                proj(ps, lambda cc: wkv_sb[:, cc * 128:(cc + 1) * 128])
                # V: copy V^T rows out, transpose per 128-block to [t, d]
                vtsb = tmp.tile([64, 512], bf16, tag="vtsb", name="vtsb")
                nc.scalar.copy(vtsb[:], ps[64:128, :])
                for half in range(2):
                    vp = psTr.tile([128, 128], bf16, tag="pTr", name="pTr")
                    for b2 in range(2):
                        b = half * 2 + b2
                        nc.tensor.transpose(
                            vp[:, b2 * 64:(b2 + 1) * 64],
                            vtsb[:, b * 128:(b + 1) * 128], identb[:64, :64])
                    tb0 = s * 4 + half * 2
                    for b2 in range(2):
                        nc.vector.tensor_copy(
                            V_all[:, (tb0 + b2) * 65:(tb0 + b2) * 65 + D],
                            vp[:, b2 * 64:(b2 + 1) * 64])
                # K: rope rows 0:64 then duplicate to 64:128 via a PE
                # identity matmul (partition shift) — avoids DMA-queue latency
                ck, sk = tab(64, s)
                rope(ps, 64, KTr[:64, scol:scol + 512], ck, sk)
                kd = psO.tile([128, 512], f32, tag="pO", name="pO")
                nc.tensor.matmul(kd[64:128, :], lhsT=identb[0:64, 0:64],
                                 rhs=KTr[:64, scol:scol + 512],
                                 start=True, stop=True)
                nc.vector.tensor_copy(KTr[64:128, scol:scol + 512],
                                      kd[64:128, :])

            def c_chunk(tb, osb, cr):
                op = psA.tile([128, 512], f32, tag="pA", name="pA")
                for oc in range(2):
                    nc.tensor.matmul(
                        op[:], lhsT=attnT[oc][:, tb * 128:(tb + 1) * 128],
                        rhs=wo_sb[oc][:, cr * 512:(cr + 1) * 512],
                        start=(oc == 0), stop=(oc == 1))
                dst = osb[:, cr * 512:(cr + 1) * 512]
                if CR_ENG[cr] == "v" and tb < 14:
                    nc.vector.tensor_copy(dst, op[:])
                else:
                    nc.scalar.copy(dst, op[:])

            def phase_b(i, tb=None, tb2=None):
                # tb: lagging output-projection row whose (always-ready)
                # matmuls are interleaved into this row's stall windows.
                # tb2: extra row emitted at the end (last-segment drain).
                if tb is not None:
                    osb = outp.tile([128, C], bf16, tag="osb", name="osb")
                b0 = max(0, i - 4)
                nj = min(i, 4) + 1
                w = nj * 128
                st_exp = {}
                horder = [0, 2, 1, 3]  # hh=0 heads first: no K-dup dependency
                for h in horder:
                    hp, hh = h // 2, h % 2
                    hoff = hh * 64
                    sp = psST.tile([128, WIN], f32, tag="pST", name="pST")
                    qs = QTr[hp][hoff:hoff + 64, i * 128:(i + 1) * 128]
                    for j in range(nj):
                        nc.tensor.matmul(
                            sp[:, j * 128:(j + 1) * 128],
                            lhsT=KTr[hoff:hoff + 64,
                                     (b0 + j) * 128:(b0 + j + 1) * 128],
                            rhs=qs, start=True, stop=True)
                    se = sexp.tile([128, WIN], bf16, tag="se", name="se")
                    nc.scalar.activation(se[:, 0:w], sp[:, 0:w], EXP)
                    if i >= 4:
                        nc.gpsimd.tensor_mul(se[:, 0:128], se[:, 0:128],
                                             maskLo)
                    nc.vector.tensor_mul(se[:, w - 128:w], se[:, w - 128:w],
                                         maskHi)
                    st_exp[h] = se
                if tb is not None:
                    c_chunk(tb, osb, 0)
                    c_chunk(tb, osb, 1)
                po = psO.tile([128, 512], f32, tag="pO", name="pO")
                # masked tiles (j=0 for i>=4, diagonal) go LAST so the PV
                # group starts as soon as exp lands, while masks apply
                if i >= 4:
                    jorder = [1, 2, 3, 0, 4]
                elif i > 0:
                    jorder = list(range(nj - 1)) + [nj - 1]
                else:
                    jorder = [0]
                for h in horder:
                    se = st_exp[h]
                    for n_, j in enumerate(jorder):
                        nc.tensor.matmul(
                            po[:, h * 65:(h + 1) * 65],
                            lhsT=se[:, j * 128:(j + 1) * 128],
                            rhs=V_all[:, (b0 + j) * 65:(b0 + j + 1) * 65],
                            start=(n_ == 0), stop=(n_ == nj - 1),
                            skip_group_check=True)
                rc = sm.tile([128, 4], f32, tag="rc", name="rc")
                nc.vector.reciprocal(rc[:], po[:, 64:260:65])
                ob = sm.tile([128, OQ], bf16, tag="obf", name="obf")
                for h in range(HQ):
                    nc.vector.tensor_scalar_mul(
                        ob[:, h * 64:(h + 1) * 64], po[:, h * 65:h * 65 + 64],
                        rc[:, h:h + 1])
                for hp in range(2):
                    # one 128x128 transpose yields both heads' [d, tq] halves
                    tp = psTr.tile([128, 128], bf16, tag="pTr", name="pTr")
                    nc.tensor.transpose(
                        tp[:], ob[:, hp * 128:(hp + 1) * 128], identb[:])
                    nc.vector.tensor_copy(
                        attnT[hp][:, i * 128:(i + 1) * 128], tp[:])
                if tb is not None:
                    c_chunk(tb, osb, 2)
                    c_chunk(tb, osb, 3)
                    nc.sync.dma_start(out=out_d[tb * 128:(tb + 1) * 128, :],
                                      in_=osb[:])
                if tb2 is not None:
                    for t2_ in tb2:
                        phase_c(t2_)

            CR_ENG = ["v", "a", "v", "v"]

            def phase_c(tb):
                osb = outp.tile([128, C], bf16, tag="osb", name="osb")
                for cr in range(4):
                    op = psA.tile([128, 512], f32, tag="pA", name="pA")
                    for oc in range(2):
                        nc.tensor.matmul(
                            op[:], lhsT=attnT[oc][:, tb * 128:(tb + 1) * 128],
                            rhs=wo_sb[oc][:, cr * 512:(cr + 1) * 512],
                            start=(oc == 0), stop=(oc == 1))
                    dst = osb[:, cr * 512:(cr + 1) * 512]
                    if cr % 2 == 0:
                        nc.vector.tensor_copy(dst, op[:])
                    else:
                        nc.scalar.copy(dst, op[:])
                    nc.sync.dma_start(
                        out=out_d[tb * 128:(tb + 1) * 128,
                                  cr * 512:(cr + 1) * 512], in_=dst)

            # ================= interleaved schedule =================
            # phase_c lags phase_b by 2 row-blocks and is emitted BEFORE the
            # b-row so its (always-ready) matmuls fill PE stalls.
            for s in range(NS):
                phase_a(s)
                for k in range(4):
                    i = s * 4 + k
                    phase_b(i, tb=i - 2 if i >= 2 else None,
                            tb2=(14,) if i == 15 else None)
                    if k == 1 and 2 <= s + 1 < NS:
                        fetch_xT(s + 1)
            phase_c(15)

            if _DEBUG:
                nc.sync.dma_start(out=dbg["dQTr0"], in_=QTr[0][:])
                nc.sync.dma_start(out=dbg["dQTr1"], in_=QTr[1][:])
                nc.sync.dma_start(out=dbg["dKTr"], in_=KTr[:])
                nc.sync.dma_start(out=dbg["dV"], in_=V_all[:])
                nc.sync.dma_start(out=dbg["dAttnT0"], in_=attnT[0][:])
                nc.sync.dma_start(out=dbg["dAttnT1"], in_=attnT[1][:])

    nc.compile()
    return nc


def _get_nc():
    if "nc" not in _cache:
        _cache["nc"] = _build()
    return _cache["nc"]


def host_inputs(x, wq, wk, wv, wo, c):
    """Pack core c's inputs into the kernel's DRAM layouts (bf16)."""
    import ml_dtypes
    bf = ml_dtypes.bfloat16
    cos2, sinS2, cosk, sinsk = _rope_tables()
    mlo, mhi = _masks()
    perm = _perm128()
    wq_c = np.asarray(wq)[:, c * OQ:(c + 1) * OQ]
    wkv_c = np.concatenate(
        [np.asarray(wk)[:, c * D:(c + 1) * D],
         np.asarray(wv)[:, c * D:(c + 1) * D]], axis=1)
    wo_c = np.asarray(wo)[c * OQ:(c + 1) * OQ, :]
    wq_c = wq_c * SCALE  # fold the 1/sqrt(d) into wq (2^-3: exact in bf16)
    wqr = wq_c.reshape(16, 128, OQ).transpose(1, 0, 2).reshape(128, 16 * OQ)
    wkvr = wkv_c.reshape(16, 128, 128).transpose(1, 0, 2).reshape(128, 16 * 128)
    wor = wo_c.reshape(2, 128, C).transpose(1, 0, 2).reshape(128, 2 * C)
    qtab = np.concatenate(
        [np.concatenate([cos2[:, s * 512:(s + 1) * 512],
                         sinS2[:, s * 512:(s + 1) * 512]], axis=1)
         for s in range(NS)], axis=1)
    pmm = np.concatenate([perm, mlo, mhi, np.eye(128)], axis=1)
    return {
        "wqr": np.ascontiguousarray(wqr).astype(bf),
        "wkvr": np.ascontiguousarray(wkvr).astype(bf),
        "wor": np.ascontiguousarray(wor).astype(bf),
        "qtab": np.ascontiguousarray(qtab).astype(bf),
        "pmm": np.ascontiguousarray(pmm).astype(bf),
    }


def kernel(x, wq, wk, wv, wo):
    from concourse.bass_utils import run_bass_kernel_spmd
    import ml_dtypes

    bf = ml_dtypes.bfloat16
    nc = _get_nc()
    x2 = np.asarray(x, dtype=np.float32).reshape(T, C)
    # pack x^T: [p, (s, h2, cc, t2)] = x[s*512 + h2*256 + t2, cc*128 + p]
    xtr = np.ascontiguousarray(
        x2.reshape(NS, 2, 256, 16, 128).transpose(4, 0, 1, 3, 2)
        .reshape(128, T * C // 128)).astype(bf)
    in_maps = []
    for c in range(NCORES):
        m = host_inputs(x, wq, wk, wv, wo, c)
        m["xtr"] = xtr
        in_maps.append(m)
    res = run_bass_kernel_spmd(nc, in_maps, list(range(NCORES)))
    out = np.zeros((T, C), dtype=np.float32)
    for r in res.results:
        out += np.asarray(r["out"], dtype=np.float32)
    return out.reshape(1, T, C)


# revision 7
# speedup vs baseline: 1.0581x; 1.0107x over previous
# Trainium2 Bass kernel for GQA with sliding-window attention (v2).
#
# B=1, T=2048, C=2048, 32 q-heads / 8 kv-heads, d_head=64, RoPE,
# sliding-window causal attention (window=512), output projection.
#
# Sharding: tensor parallel over heads across 8 cores. Core c owns q-heads
# [4c, 4c+4) and kv-head c; computes the partial output
# attn_out_shard @ wo[256c:256(c+1), :] in bf16; host sums the 8 partials.
#
# v2 strategy (vs baseline): all-bf16 data paths with fp32 PSUM accumulation;
# x^T produced by hardware DMA-transpose (no PE transposes, no SBUF staging
# copies); scores computed transposed (ST[tk,tq]) so softmax P needs no
# transposition before PV; PV computes O[tq,d] with row-sums accumulated by
# ones-matmuls so normalization is a per-partition tensor_scalar; the 1/8
# scale is folded into the Q RoPE tables; phases A (proj+rope), B (attention)
# and C (output proj) are emission-interleaved per 512-row superblock so
# PE/ACT/DVE/Pool/DMA all stay busy.

import numpy as np

T = 2048
C = 2048
N_HEADS = 32
N_KV = 8
D = 64
WINDOW = 512
NCORES = 8
HQ = N_HEADS // NCORES          # 4 q heads per core
OQ = HQ * D                     # 256
ROPE_BASE = 10000.0
SCALE = 1.0 / 8.0               # 1/sqrt(64)
NB = T // 128                   # 16 row blocks
NS = T // 512                   # 4 superblocks
WIN = 640                       # max key window width per row block

_cache = {}
_DEBUG = False


def _rope_tables():
    inv = 1.0 / (ROPE_BASE ** (np.arange(0, D, 2, dtype=np.float64) / D))
    t = np.arange(T, dtype=np.float64)
    fr = t[:, None] * inv[None, :]            # [T, 32]
    emb = np.concatenate([fr, fr], axis=1)    # [T, 64]
    cos = np.cos(emb).T                       # [64, T]
    sin = np.sin(emb).T
    sinS = sin.copy()
    sinS[: D // 2] *= -1.0                    # signed sin for rotate_half
    cos2 = np.concatenate([cos, cos], axis=0)     # [128, T] (2 heads/tile)
    sinS2 = np.concatenate([sinS, sinS], axis=0)  # [128, T]
    return cos2, sinS2, cos, sinS


def _perm128():
    p = np.zeros((128, 128), dtype=np.float64)
    for s in range(128):
        blk = (s // 64) * 64
        d = s - blk
        p[s, blk + (d + 32) % 64] = 1.0
    return p


def _masks():
    r = np.arange(128)[:, None]
    c = np.arange(128)[None, :]
    lo = (c <= r).astype(np.float64)   # ST j=0 tile (i>=4): allowed c<=r
    hi = (c >= r).astype(np.float64)   # ST diagonal tile: allowed c>=r
    return lo, hi


def _build():
    import concourse.bacc as bacc
    import concourse.mybir as mybir
    import concourse.tile as tile

    f32 = mybir.dt.float32
    bf16 = mybir.dt.bfloat16
    EXP = mybir.ActivationFunctionType.Exp

    nc = bacc.Bacc("TRN2", target_bir_lowering=False, debug=False,
                   num_devices=NCORES)

    # x^T host-packed as [128, s*8192 + h2*4096 + cc*256 + t2] so each
    # half-superblock of x^T loads with one contiguous DMA.
    x_d = nc.dram_tensor("xtr", [128, T * C // 128], bf16,
                         kind="ExternalInput").ap()
    # host-packed weights: [128, n] layouts so each loads with ONE DMA
    wq_d = nc.dram_tensor("wqr", [128, 16 * OQ], bf16, kind="ExternalInput").ap()
    wkv_d = nc.dram_tensor("wkvr", [128, 16 * 128], bf16,
                           kind="ExternalInput").ap()
    wo_d = nc.dram_tensor("wor", [128, 2 * C], bf16, kind="ExternalInput").ap()
    qtab_d = nc.dram_tensor("qtab", [128, 2 * T], bf16,
                            kind="ExternalInput").ap()
    pmm_d = nc.dram_tensor("pmm", [128, 4 * 128], bf16,
                           kind="ExternalInput").ap()
    out_d = nc.dram_tensor("out", [T, C], bf16, kind="ExternalOutput").ap()
    dbg = {}
    if _DEBUG:
        for nm, shp in [("dQTr0", [128, T]), ("dQTr1", [128, T]),
                        ("dKTr", [128, T]), ("dV", [128, NB * 65]),
                        ("dAttnT0", [128, T]), ("dAttnT1", [128, T])]:
            dbg[nm] = nc.dram_tensor(nm, shp, bf16, kind="ExternalOutput").ap()

    with tile.TileContext(nc) as tc:
        from contextlib import ExitStack
        ctx = ExitStack()
        with ctx:
            const = ctx.enter_context(tc.tile_pool(name="const", bufs=1))
            persist = ctx.enter_context(tc.tile_pool(name="persist", bufs=1))

            tmp = ctx.enter_context(tc.tile_pool(name="tmp", bufs=4))
            sm = ctx.enter_context(tc.tile_pool(name="small", bufs=6))
            sexp = ctx.enter_context(tc.tile_pool(name="sexp", bufs=6))
            outp = ctx.enter_context(tc.tile_pool(name="outp", bufs=4))
            psA = ctx.enter_context(
                tc.tile_pool(name="psA", bufs=2, space="PSUM"))
            psST = ctx.enter_context(
                tc.tile_pool(name="psST", bufs=2, space="PSUM"))
            psO = ctx.enter_context(
                tc.tile_pool(name="psO", bufs=1, space="PSUM"))
            psTr = ctx.enter_context(
                tc.tile_pool(name="psTr", bufs=1, space="PSUM"))

            # ---- constants / weights into SBUF (one DMA each) ----
            from concourse.masks import make_identity

            wq_sb = const.tile([128, 16 * OQ], bf16, tag="wq", name="wq")

            xTr = const.tile([128, T * C // 128], bf16, tag="xTr", name="xTr")

            def fetch_xT(s):
                for h2 in range(2):
                    off = s * 8192 + h2 * 4096
                    nc.sync.dma_start(out=xTr[:, off:off + 4096],
                                      in_=x_d[:, off:off + 4096])

            def xT_slice(s, h2, cc):
                off = s * 8192 + h2 * 4096 + cc * 256
                return xTr[:, off:off + 256]

            # startup: interleave wq chunks with quarter-loads of x^T(s0)
            nc.sync.dma_start(out=wq_sb[:, 0:4 * OQ], in_=wq_d[:, 0:4 * OQ])
            nc.sync.dma_start(out=xTr[:, 0:2048], in_=x_d[:, 0:2048])
            nc.sync.dma_start(out=wq_sb[:, 4 * OQ:8 * OQ],
                              in_=wq_d[:, 4 * OQ:8 * OQ])
            nc.sync.dma_start(out=xTr[:, 2048:4096], in_=x_d[:, 2048:4096])
            nc.sync.dma_start(out=wq_sb[:, 8 * OQ:12 * OQ],
                              in_=wq_d[:, 8 * OQ:12 * OQ])
            nc.sync.dma_start(out=xTr[:, 4096:6144], in_=x_d[:, 4096:6144])
            nc.sync.dma_start(out=wq_sb[:, 12 * OQ:16 * OQ],
                              in_=wq_d[:, 12 * OQ:16 * OQ])
            nc.sync.dma_start(out=xTr[:, 6144:8192], in_=x_d[:, 6144:8192])
            wkv_sb = const.tile([128, 16 * 128], bf16, tag="wkv", name="wkv")
            nc.sync.dma_start(out=wkv_sb[:], in_=wkv_d[:, :])
            qtab = const.tile([128, 2 * T], bf16, tag="qtab", name="qtab")
            nc.sync.dma_start(out=qtab[:, 0:1024], in_=qtab_d[:, 0:1024])
            pmm = const.tile([128, 4 * 128], bf16, tag="pmm", name="pmm")
            nc.sync.dma_start(out=pmm[:], in_=pmm_d[:, :])
            nc.sync.dma_start(out=qtab[:, 1024:2048], in_=qtab_d[:, 1024:2048])
            wo_sb2 = const.tile([128, 2 * C], bf16, tag="wo", name="wo")
            nc.sync.dma_start(out=wo_sb2[:, 0:C], in_=wo_d[:, 0:C])
            nc.sync.dma_start(out=qtab[:, 2048:3072], in_=qtab_d[:, 2048:3072])
            nc.sync.dma_start(out=wo_sb2[:, C:2 * C], in_=wo_d[:, C:2 * C])
            nc.sync.dma_start(out=qtab[:, 3072:4096], in_=qtab_d[:, 3072:4096])
            fetch_xT(1)

            # wq is pre-scaled by 1/8 on the host, so the K rope tables are
            # just the first 64 rows of the (unscaled) Q tables.
            def tab(P, s):
                # per-superblock packed tables: [s*1024 + (cos 512 | sin 512)]
                return (qtab[0:P, s * 1024:s * 1024 + 512],
                        qtab[0:P, s * 1024 + 512:s * 1024 + 1024])
            perm = pmm[:, 0:128]
            maskLo = pmm[:, 128:256]
            maskHi = pmm[:, 256:384]
            wo_sb = [wo_sb2[:, 0:C], wo_sb2[:, C:2 * C]]

            identb = pmm[:, 384:512]

            # ---- persistent activations ----
            QTr = [persist.tile([128, T], bf16, tag=f"QTr{hp}", name=f"QTr{hp}")
                   for hp in range(2)]
            KTr = persist.tile([128, T], bf16, tag="KTr", name="KTr")
            # V blocks interleaved with a ones column: [V_b | 1] of width 65
            # per 128-row block, so PV row-sums come from the same matmul.
            V_all = persist.tile([128, NB * 65], bf16, tag="V", name="V")
            nc.vector.memset(V_all[:], 1.0)
            attnT = [persist.tile([128, T], bf16, tag=f"attnT{oc}",
                                  name=f"attnT{oc}") for oc in range(2)]

            def rope(ps, P, dst, cos_t, sin_t):
                # dst = ps*cos + rot(ps)*sinS, written as bf16.
                # rot via perm matmul on PE into a separate PSUM tile so the
                # chain is qraw -> rot -> t2 -> add with t1 off-path.
                qraw = tmp.tile([128, 512], bf16, tag="qraw", name="qraw")
                nc.scalar.copy(qraw[:P, :], ps[:P, :])
                t1 = tmp.tile([128, 512], bf16, tag="rt1", name="rt1")
                nc.gpsimd.tensor_mul(t1[:P, :], qraw[:P, :], cos_t)
                rot = psO.tile([128, 512], f32, tag="pO", name="pO")
                nc.tensor.matmul(rot[:P, :], lhsT=perm[:P, :P],
                                 rhs=qraw[:P, :], start=True, stop=True)
                t2 = tmp.tile([128, 512], bf16, tag="rt2", name="rt2")
                nc.vector.tensor_mul(t2[:P, :], rot[:P, :], sin_t)
                nc.vector.tensor_add(dst, t1[:P, :], t2[:P, :])

            def phase_a(s):
                scol = s * 512
                def proj(ps, lhs_of):
                    # contract C in 16 chunks, two 256-wide t-halves so the
                    # first half can start before the second transpose lands
                    for h2 in range(2):
                        for cc in range(16):
                            nc.tensor.matmul(
                                ps[:, h2 * 256:(h2 + 1) * 256],
                                lhsT=lhs_of(cc),
                                rhs=xT_slice(s, h2, cc),
                                start=(cc == 0), stop=(cc == 15))

                # Q projections: 2 head-pair blocks of 128 out dims
                for ob in range(2):
                    ps = psA.tile([128, 512], f32, tag="pA", name="pA")
                    proj(ps, lambda cc: wq_sb[:, cc * OQ + ob * 128:
                                              cc * OQ + (ob + 1) * 128])
                    cq, sq = tab(128, s)
                    rope(ps, 128, QTr[ob][:, scol:scol + 512], cq, sq)
                # K+V packed projection: rows 0:64 = K^T, 64:128 = V^T
                ps = psA.tile([128, 512], f32, tag="pA", name="pA")
                proj(ps, lambda cc: wkv_sb[:, cc * 128:(cc + 1) * 128])
                # V: copy V^T rows out, transpose per 128-block to [t, d]
                vtsb = tmp.tile([64, 512], bf16, tag="vtsb", name="vtsb")
                nc.scalar.copy(vtsb[:], ps[64:128, :])
                for half in range(2):
                    vp = psTr.tile([128, 128], bf16, tag="pTr", name="pTr")
                    for b2 in range(2):
                        b = half * 2 + b2
                        nc.tensor.transpose(
                            vp[:, b2 * 64:(b2 + 1) * 64],
                            vtsb[:, b * 128:(b + 1) * 128], identb[:64, :64])
                    tb0 = s * 4 + half * 2
                    for b2 in range(2):
                        nc.vector.tensor_copy(
                            V_all[:, (tb0 + b2) * 65:(tb0 + b2) * 65 + D],
                            vp[:, b2 * 64:(b2 + 1) * 64])
                # K: rope rows 0:64 then duplicate to 64:128 via a PE
                # identity matmul (partition shift) — avoids DMA-queue latency
                ck, sk = tab(64, s)
                rope(ps, 64, KTr[:64, scol:scol + 512], ck, sk)
                kd = psO.tile([128, 512], f32, tag="pO", name="pO")
                nc.tensor.matmul(kd[64:128, :], lhsT=identb[0:64, 0:64],
                                 rhs=KTr[:64, scol:scol + 512],
                                 start=True, stop=True)
                nc.vector.tensor_copy(KTr[64:128, scol:scol + 512],
                                      kd[64:128, :])

            def c_chunk(tb, osb, cr):
                op = psA.tile([128, 512], f32, tag="pA", name="pA")
                for oc in range(2):
                    nc.tensor.matmul(
                        op[:], lhsT=attnT[oc][:, tb * 128:(tb + 1) * 128],
                        rhs=wo_sb[oc][:, cr * 512:(cr + 1) * 512],
                        start=(oc == 0), stop=(oc == 1))
                dst = osb[:, cr * 512:(cr + 1) * 512]
                if CR_ENG[cr] == "v" and tb < 14:
                    nc.vector.tensor_copy(dst, op[:])
                else:
                    nc.scalar.copy(dst, op[:])

            def phase_b(i, tb=None, tb2=None):
                # tb: lagging output-projection row whose (always-ready)
                # matmuls are interleaved into this row's stall windows.
                # tb2: extra row emitted at the end (last-segment drain).
                if tb is not None:
                    osb = outp.tile([128, C], bf16, tag="osb", name="osb")
                b0 = max(0, i - 4)
                nj = min(i, 4) + 1
                w = nj * 128
                st_exp = {}
                horder = [0, 2, 1, 3]  # hh=0 heads first: no K-dup dependency
                for h in horder:
                    hp, hh = h // 2, h % 2
                    hoff = hh * 64
                    sp = psST.tile([128, WIN], f32, tag="pST", name="pST")
                    qs = QTr[hp][hoff:hoff + 64, i * 128:(i + 1) * 128]
                    for j in range(nj):
                        nc.tensor.matmul(
                            sp[:, j * 128:(j + 1) * 128],
                            lhsT=KTr[hoff:hoff + 64,
                                     (b0 + j) * 128:(b0 + j + 1) * 128],
                            rhs=qs, start=True, stop=True)
                    se = sexp.tile([128, WIN], bf16, tag="se", name="se")
                    nc.scalar.activation(se[:, 0:w], sp[:, 0:w], EXP)
                    if i >= 4:
                        nc.gpsimd.tensor_mul(se[:, 0:128], se[:, 0:128],
                                             maskLo)
                    nc.vector.tensor_mul(se[:, w - 128:w], se[:, w - 128:w],
                                         maskHi)
                    st_exp[h] = se
                if tb is not None:
                    c_chunk(tb, osb, 0)
                    c_chunk(tb, osb, 1)
                po = psO.tile([128, 512], f32, tag="pO", name="pO")
                # masked tiles (j=0 for i>=4, diagonal) go LAST so the PV
                # group starts as soon as exp lands, while masks apply
                if i >= 4:
                    jorder = [1, 2, 3, 4, 0]
                elif i > 0:
                    jorder = list(range(nj - 1)) + [nj - 1]
                else:
                    jorder = [0]
                for h in horder:
                    se = st_exp[h]
                    for n_, j in enumerate(jorder):
                        nc.tensor.matmul(
                            po[:, h * 65:(h + 1) * 65],
                            lhsT=se[:, j * 128:(j + 1) * 128],
                            rhs=V_all[:, (b0 + j) * 65:(b0 + j + 1) * 65],
                            start=(n_ == 0), stop=(n_ == nj - 1),
                            skip_group_check=True)
                rc = sm.tile([128, 4], f32, tag="rc", name="rc")
                nc.vector.reciprocal(rc[:], po[:, 64:260:65])
                ob = sm.tile([128, OQ], bf16, tag="obf", name="obf")
                for h in range(HQ):
                    nc.vector.tensor_scalar_mul(
                        ob[:, h * 64:(h + 1) * 64], po[:, h * 65:h * 65 + 64],
                        rc[:, h:h + 1])
                for hp in range(2):
                    # one 128x128 transpose yields both heads' [d, tq] halves
                    tp = psTr.tile([128, 128], bf16, tag="pTr", name="pTr")
                    nc.tensor.transpose(
                        tp[:], ob[:, hp * 128:(hp + 1) * 128], identb[:])
                    nc.vector.tensor_copy(
                        attnT[hp][:, i * 128:(i + 1) * 128], tp[:])
                if tb is not None:
                    c_chunk(tb, osb, 2)
                    c_chunk(tb, osb, 3)
                    nc.sync.dma_start(out=out_d[tb * 128:(tb + 1) * 128, :],
                                      in_=osb[:])
                if tb2 is not None:
                    for t2_ in tb2:
                        phase_c(t2_)

            CR_ENG = ["v", "a", "v", "v"]

            def phase_c(tb):
                osb = outp.tile([128, C], bf16, tag="osb", name="osb")
                for cr in range(4):
                    op = psA.tile([128, 512], f32, tag="pA", name="pA")
                    for oc in range(2):
                        nc.tensor.matmul(
                            op[:], lhsT=attnT[oc][:, tb * 128:(tb + 1) * 128],
                            rhs=wo_sb[oc][:, cr * 512:(cr + 1) * 512],
                            start=(oc == 0), stop=(oc == 1))
                    dst = osb[:, cr * 512:(cr + 1) * 512]
                    if cr % 2 == 0:
                        nc.vector.tensor_copy(dst, op[:])
                    else:
                        nc.scalar.copy(dst, op[:])
                    nc.sync.dma_start(
                        out=out_d[tb * 128:(tb + 1) * 128,
                                  cr * 512:(cr + 1) * 512], in_=dst)

            # ================= interleaved schedule =================
            # phase_c lags phase_b by 2 row-blocks and is emitted BEFORE the
            # b-row so its (always-ready) matmuls fill PE stalls.
            for s in range(NS):
                phase_a(s)
                for k in range(4):
                    i = s * 4 + k
                    phase_b(i, tb=i - 2 if i >= 2 else None,
                            tb2=(14,) if i == 15 else None)
                    if k == 1 and 2 <= s + 1 < NS:
                        fetch_xT(s + 1)
            phase_c(15)

            if _DEBUG:
                nc.sync.dma_start(out=dbg["dQTr0"], in_=QTr[0][:])
                nc.sync.dma_start(out=dbg["dQTr1"], in_=QTr[1][:])
                nc.sync.dma_start(out=dbg["dKTr"], in_=KTr[:])
                nc.sync.dma_start(out=dbg["dV"], in_=V_all[:])
                nc.sync.dma_start(out=dbg["dAttnT0"], in_=attnT[0][:])
                nc.sync.dma_start(out=dbg["dAttnT1"], in_=attnT[1][:])

    nc.compile()
    return nc


def _get_nc():
    if "nc" not in _cache:
        _cache["nc"] = _build()
    return _cache["nc"]


def host_inputs(x, wq, wk, wv, wo, c):
    """Pack core c's inputs into the kernel's DRAM layouts (bf16)."""
    import ml_dtypes
    bf = ml_dtypes.bfloat16
    cos2, sinS2, cosk, sinsk = _rope_tables()
    mlo, mhi = _masks()
    perm = _perm128()
    wq_c = np.asarray(wq)[:, c * OQ:(c + 1) * OQ]
    wkv_c = np.concatenate(
        [np.asarray(wk)[:, c * D:(c + 1) * D],
         np.asarray(wv)[:, c * D:(c + 1) * D]], axis=1)
    wo_c = np.asarray(wo)[c * OQ:(c + 1) * OQ, :]
    wq_c = wq_c * SCALE  # fold the 1/sqrt(d) into wq (2^-3: exact in bf16)
    wqr = wq_c.reshape(16, 128, OQ).transpose(1, 0, 2).reshape(128, 16 * OQ)
    wkvr = wkv_c.reshape(16, 128, 128).transpose(1, 0, 2).reshape(128, 16 * 128)
    wor = wo_c.reshape(2, 128, C).transpose(1, 0, 2).reshape(128, 2 * C)
    qtab = np.concatenate(
        [np.concatenate([cos2[:, s * 512:(s + 1) * 512],
                         sinS2[:, s * 512:(s + 1) * 512]], axis=1)
         for s in range(NS)], axis=1)
    pmm = np.concatenate([perm, mlo, mhi, np.eye(128)], axis=1)
    return {
        "wqr": np.ascontiguousarray(wqr).astype(bf),
        "wkvr": np.ascontiguousarray(wkvr).astype(bf),
        "wor": np.ascontiguousarray(wor).astype(bf),
        "qtab": np.ascontiguousarray(qtab).astype(bf),
        "pmm": np.ascontiguousarray(pmm).astype(bf),
    }


def kernel(x, wq, wk, wv, wo):
    from concourse.bass_utils import run_bass_kernel_spmd
    import ml_dtypes

    bf = ml_dtypes.bfloat16
    nc = _get_nc()
    x2 = np.asarray(x, dtype=np.float32).reshape(T, C)
    # pack x^T: [p, (s, h2, cc, t2)] = x[s*512 + h2*256 + t2, cc*128 + p]
    xtr = np.ascontiguousarray(
        x2.reshape(NS, 2, 256, 16, 128).transpose(4, 0, 1, 3, 2)
        .reshape(128, T * C // 128)).astype(bf)
    in_maps = []
    for c in range(NCORES):
        m = host_inputs(x, wq, wk, wv, wo, c)
        m["xtr"] = xtr
        in_maps.append(m)
    res = run_bass_kernel_spmd(nc, in_maps, list(range(NCORES)))
    out = np.zeros((T, C), dtype=np.float32)
    for r in res.results:
        out += np.asarray(r["out"], dtype=np.float32)
    return out.reshape(1, T, C)
